# revision 17
# baseline (speedup 1.0000x reference)
"""Trainium2 Bass kernel for CosmicMultiHeadAttention (block-local flash attention).

Sharding: the 8192 tokens (B=2 x S=4096) are split into 8 shards of 1024
tokens (batch-major). Attention is block-local with 128-token blocks, so
1024-token shards (8 blocks each) have zero cross-shard dependencies: every
core runs the full layer (QKV proj + RoPE + block attention + out proj) for
its own tokens. No collectives.

v2 layout strategy (single pass over all 1024 tokens, weights loaded once):
  - x is transposed on the host to xT [E, tok] so the E contraction sits on
    the partition axis; resident in SBUF for the whole projection phase.
  - V proj first: its ko-loop matmuls are paced by the arriving xt DMAs, so
    the PE starts ~3us in and stays busy through the initial load.
    Swapped operands (lhsT = xt tiles) land v natural [tok, hd] - exactly
    the PV-matmul lhsT layout.
  - K/Q projections: lhsT = weight tiles (1MB DMA each, loaded ONCE),
    rhs = xt -> psum kT/qT [hd, tok]. RoPE applied during psum eviction
    (ACT copy + partition-shift DMA + DVE).
  - attention: 64 iterations (8 blocks x 8 kv-heads, 4 grouped q-heads,
    N=512), 3-stage software pipeline across engines:
      stage1(i):   s = kT.T @ qT;  ACT exp (scale, mask-bias) -> wT (SBUF)
      stage2(i-1): l = ones.T @ wT; DVE recip; pv = v.T @ wT
      stage3(i-2): bc = broadcast(1/l) via K=1 matmul; DVE mul -> oT
    PE order per step: s(i), l(i-1), pv(i-1), bc(i-2) - every operand was
    produced >= 1 full step (~850ns) earlier, so the PE never waits on the
    ACT-exp / DVE-recip chains and HAM stays at K=8/8.
  - out projection: lhsT = oT tiles (on-chip), rhs = wo tiles (loaded once),
    8 psum banks accumulate over the 32 head k-tiles, evictions alternate
    ACT/DVE so the next n-tile's first matmuls aren't serialized behind a
    single engine.
"""

import sys

if '/opt/trn_rl_repo' not in sys.path:
    sys.path.insert(0, '/opt/trn_rl_repo')

import numpy as np
import ml_dtypes

import concourse.bass as bass
import concourse.tile as tile
from concourse import mybir
from concourse.bass_utils import run_bass_kernel_spmd

BF16 = mybir.dt.bfloat16
F32 = mybir.dt.float32
NPBF16 = ml_dtypes.bfloat16

B, S, E = 2, 4096, 4096
HQ, HKV, D = 32, 8, 128
BS = 128
ROPE_BASE = 10000.0
NCORES = 8
TOK = (B * S) // NCORES          # 1024 tokens per core
KO = E // 128                    # 32 k-tiles over E
MQ = (HQ * D) // 128             # 32 q head-tiles
MK = (HKV * D) // 128            # 8 k head-tiles
G = HQ // HKV                    # 4 q heads per kv head
NBLK = TOK // BS                 # 8 blocks per core
NE = E // 512                    # 8 out-proj column tiles
SCALE = 1.0 / float(np.sqrt(D))


# ---------------------------------------------------------------------------
# The walrus build in this image rejects instructions carrying more than one
# "sem-ge" sync wait ("Too many sync wait commands"; Drain/CTRL accepts
# none). Tile's scheduler freely attaches several. Post-pass: keep at most
# one ge-wait per instruction (none on Drain) and move the excess onto
# EventSemaphore carrier instructions inserted immediately before, on the
# same engine - program order preserves the blocking semantics exactly.
# ---------------------------------------------------------------------------
def _split_excess_waits(nc):
    import bass_rust
    ctr = 0
    for f in nc.m.functions:
        for bb in f.blocks:
            out_list = []
            for inst in bb.instructions:
                si = inst.sync_info
                all_waits = list(si.on_wait) if si and si.on_wait else []
                ge = [w for w in all_waits if 'ge' in w.wait_mode]
                eq = [w for w in all_waits if 'ge' not in w.wait_mode]
                keep_n = 0 if type(inst).__name__ == 'InstDrain' else 1
                if len(ge) > keep_n:
                    extra, keep = ge[:-keep_n] if keep_n else ge, \
                        ge[-keep_n:] if keep_n else []
                    for w in extra:
                        ctr += 1
                        es = mybir.InstEventSemaphore(
                            name=f'waitsplit_{ctr}', engine=inst.engine,
                            ins=[], outs=[],
                            sync_info=bass_rust.SyncInfo(
                                on_wait=[w], on_update=[]))
                        out_list.append(es)
                    si.on_wait = eq + keep
                out_list.append(inst)
            bb.instructions[:] = out_list
    return nc


def _act_recip(nc, out_ap, in_ap):
    # ACT LUT reciprocal, emitted directly: the bass wrapper refuses
    # Reciprocal on accuracy grounds, but ~1e-3 relative is ample here and
    # every DVE reciprocal path is either ~6 passes (3.3us per call) or
    # unsupported by this walrus build.
    se = nc.scalar
    ins = [se.lower_ap(in_ap)]
    for arg in (0.0, 1.0, 0.0):  # bias, scale, alpha
        ins.append(mybir.ImmediateValue(dtype=mybir.dt.float32, value=arg))
    return se.add_instruction(
        mybir.InstActivation(
            name=se.bass.get_next_instruction_name(),
            func=mybir.ActivationFunctionType.Reciprocal,
            ins=ins, outs=[se.lower_ap(out_ap)]))


def _build(use_bias: bool):
    nc = bass.Bass()

    xT = nc.dram_tensor("xT", [KO, 128, TOK], BF16, kind="ExternalInput")
    wq_t = nc.dram_tensor("wq_t", [MQ, 128, KO, 128], BF16, kind="ExternalInput")
    wk_t = nc.dram_tensor("wk_t", [MK, 128, KO, 128], BF16, kind="ExternalInput")
    wv_t = nc.dram_tensor("wv_t", [KO, 2, 128, 512], BF16, kind="ExternalInput")
    wo_t = nc.dram_tensor("wo_t", [MQ, NE, 128, 512], BF16, kind="ExternalInput")
    cos_t = nc.dram_tensor("cos_t", [128, TOK], BF16, kind="ExternalInput")
    sin_t = nc.dram_tensor("sin_t", [128, TOK], BF16, kind="ExternalInput")
    mb_t = nc.dram_tensor("mb_t", [NBLK, 128], F32, kind="ExternalInput")
    if use_bias:
        bq_t = nc.dram_tensor("bq_t", [MQ, 128], F32, kind="ExternalInput")
        bk_t = nc.dram_tensor("bk_t", [MK, 128], F32, kind="ExternalInput")
        bv_t = nc.dram_tensor("bv_t", [2, 512], BF16, kind="ExternalInput")
        bo_t = nc.dram_tensor("bo_t", [NE, 512], BF16, kind="ExternalInput")
    out = nc.dram_tensor("out", [TOK, E], BF16, kind="ExternalOutput")

    with tile.TileContext(nc) as tc:
        with (
            tc.tile_pool(name="const", bufs=1) as cpool,
            tc.tile_pool(name="keep", bufs=1) as keep,
        ):
            # ---- constants (DMAs issued after xt below; xt is critical path) ----
            cos_sb = cpool.tile([128, TOK], BF16, tag="cos")
            sin_sb = cpool.tile([128, TOK], BF16, tag="sin")
            mb_sb = cpool.tile([128, NBLK], F32, tag="mb")
            ones_col = cpool.tile([128, 1], BF16, tag="ones_col")
            nc.vector.memset(ones_col[:], 1.0)
            ones_row_f = cpool.tile([64, 128], F32, tag="ones_row_f")
            nc.vector.memset(ones_row_f[:], 1.0)
            if use_bias:
                ones_row = cpool.tile([1, 128], BF16, tag="ones_row")
                nc.vector.memset(ones_row[:], 1.0)
                bq_sb = cpool.tile([128, MQ], F32, tag="bq")
                nc.sync.dma_start(bq_sb[:], bq_t.rearrange("m p -> p m"))
                bk_sb = cpool.tile([128, MK], F32, tag="bk")
                nc.sync.dma_start(bk_sb[:], bk_t.rearrange("m p -> p m"))
                bv_sb = cpool.tile([1, 2, 512], BF16, tag="bv")
                nc.sync.dma_start(bv_sb[:], bv_t[None, :, :])
                bo_sb = cpool.tile([1, NE, 512], BF16, tag="bo")
                nc.sync.dma_start(bo_sb[:], bo_t[None, :, :])

            # persistent intermediates
            qT = keep.tile([128, MQ, TOK], BF16, tag="qT")
            kT = keep.tile([128, MK, TOK], BF16, tag="kT")
            v_sb = keep.tile([128, NBLK, HKV * D], BF16, tag="v")

            # ================= projection phase =================
            with (
                tc.tile_pool(name="xtp", bufs=1) as xt_pool,
                tc.tile_pool(name="wqk", bufs=2) as wqk_pool,
                tc.tile_pool(name="wmov", bufs=3) as wmov_pool,
                tc.tile_pool(name="rope", bufs=2) as rope_pool,
            ):
                # fan the startup DMA dispatch across four idle engine
                # queues: one engine issues ~300ns/dma_start, which alone
                # adds ~10us before all 16 DMA engines have work.
                xt = xt_pool.tile([128, KO, TOK], BF16, tag="xt")
                _eng = [nc.sync, nc.scalar, nc.gpsimd]
                for ko in range(KO):
                    _eng[ko % 3].dma_start(xt[:, ko, :], xT[ko])
                nc.sync.dma_start(cos_sb[:], cos_t[:, :])
                nc.scalar.dma_start(sin_sb[:], sin_t[:, :])
                nc.gpsimd.dma_start(mb_sb[:], mb_t.rearrange("b p -> p b"))

                # ---- V projection (paced by xt arrival) ----
                with tc.tile_pool(name="psv", bufs=8, space="PSUM") as psv:
                    for n in range(2):
                        pss = [psv.tile([128, 512], F32, tag="ps",
                                        name=f"psv{n}_{mt}")
                               for mt in range(NBLK)]
                        for ko in range(KO):
                            wv_sb = wmov_pool.tile([128, 512], BF16, tag="wv")
                            (nc.gpsimd if ko % 2 else nc.sync).dma_start(
                                wv_sb[:], wv_t[ko, n])
                            for mt in range(NBLK):
                                nc.tensor.matmul(
                                    pss[mt][:],
                                    xt[:, ko, mt * 128:(mt + 1) * 128],
                                    wv_sb[:], start=(ko == 0),
                                    stop=(ko == KO - 1 and not use_bias))
                        if use_bias:
                            for mt in range(NBLK):
                                nc.tensor.matmul(pss[mt][:], ones_row[:],
                                                 bv_sb[:, n, :],
                                                 start=False, stop=True)
                        for mt in range(NBLK):
                            dst = v_sb[:, mt, n * 512:(n + 1) * 512]
                            nc.scalar.copy(dst[:, 0:256], pss[mt][:, 0:256])
                            nc.vector.tensor_copy(dst[:, 256:512],
                                                  pss[mt][:, 256:512])

                # ---- K/Q projections with fused RoPE eviction ----
                def rope_evict(ps, dst_ap, h, m, bias_sb):
                    ts = slice(h * 512, (h + 1) * 512)
                    qa = rope_pool.tile([128, 512], BF16, tag="qa")
                    if bias_sb is not None:
                        nc.scalar.add(qa[:], ps[:], bias_sb[:, m:m + 1])
                    else:
                        nc.scalar.copy(qa[:], ps[:])
                    qsh = rope_pool.tile([128, 512], BF16, tag="qsh")
                    nc.sync.dma_start(qsh[0:64, :], qa[64:128, :])
                    nc.sync.dma_start(qsh[64:128, :], qa[0:64, :])
                    t1 = rope_pool.tile([128, 512], BF16, tag="t1")
                    nc.vector.tensor_mul(t1[:], qa[:], cos_sb[:, ts])
                    t2 = rope_pool.tile([128, 512], BF16, tag="t2")
                    nc.vector.tensor_mul(t2[:], qsh[:], sin_sb[:, ts])
                    nc.vector.tensor_add(dst_ap, t1[:], t2[:])

                # ---- K proj, then Q proj with woven attention ----
                # The attention pipeline rides inside the Q-projection
                # stream: one attention step is emitted after each Q psum
                # chain (64 chains <-> 64 steps, offset 9 so group g's
                # attention runs while group g+1 projects). Every attention
                # operand is then ~7us old when the PE reaches it, and the
                # ACT Exp<->Reciprocal table reloads (1.3us each) hide
                # entirely behind projection matmuls. Attention output is
                # normalized in-place into qT (each slice is dead as a query
                # once its s-matmul has run), which keeps SBUF under budget.
                with (
                    tc.tile_pool(name="psqk", bufs=2, space="PSUM") as psqk,
                    tc.tile_pool(name="ps_s", bufs=2, space="PSUM") as ps_s,
                    tc.tile_pool(name="ps_l", bufs=2, space="PSUM") as ps_l,
                    tc.tile_pool(name="ps_pv", bufs=1, space="PSUM") as ps_pv,
                    tc.tile_pool(name="ps_bc", bufs=1, space="PSUM") as ps_bc,
                    tc.tile_pool(name="attn", bufs=6) as attn_pool,
                    tc.tile_pool(name="bcbp", bufs=2) as bcb_pool,
                    tc.tile_pool(name="rcp", bufs=3) as rc_pool,
                ):
                    for m in range(MK):
                        wsb = wqk_pool.tile([128, KO, 128], BF16, tag="w")
                        nc.sync.dma_start(wsb[:], wk_t[m])
                        for h in range(2):
                            ps = psqk.tile([128, 512], F32, tag="ps")
                            for ko in range(KO):
                                nc.tensor.matmul(
                                    ps[:], wsb[:, ko, :],
                                    xt[:, ko, h * 512:(h + 1) * 512],
                                    start=(ko == 0), stop=(ko == KO - 1))
                            rope_evict(ps, kT[:, m, h * 512:(h + 1) * 512],
                                       h, m, bk_sb if use_bias else None)

                    iters = [(g, blk) for g in range(HKV)
                             for blk in range(NBLK)]
                    NIT = len(iters)
                    LAG = 5
                    st_wT = {}
                    st_rc = {}
                    l_cur = {}

                    def attn_step(i):
                        # stage A: scores + exp for iteration i
                        if i < NIT:
                            g, blk = iters[i]
                            tq = slice(blk * 128, (blk + 1) * 128)
                            s_ps = ps_s.tile([128, 512], F32, tag="s")
                            nc.tensor.matmul(
                                s_ps[:], kT[:, g, tq],
                                qT[:, g * G:(g + 1) * G, tq],
                                start=True, stop=True)
                            wT = attn_pool.tile([128, 512], BF16, tag="wT")
                            nc.scalar.activation(
                                out=wT[:], in_=s_ps[:],
                                func=mybir.ActivationFunctionType.Exp,
                                scale=SCALE, bias=mb_sb[:, blk:blk + 1])
                            st_wT[i] = wT
                        # stage B: row-sum; adjacent reciprocal pair per 4
                        j = i - 1
                        if 0 <= j < NIT:
                            b, r = divmod(j, 2)
                            if r == 0:
                                l_cur[b] = ps_l.tile([128, 512], F32,
                                                     tag="l", name=f"l{b}")
                            nc.tensor.matmul(
                                l_cur[b][32 * r:32 * r + 1, :], ones_col[:],
                                st_wT[j][:], start=True, stop=True)
                            if r == 1 and (b % 2 == 1 or j == NIT - 1):
                                for bb in ([b - 1, b] if b % 2 == 1 else [b]):
                                    rc = rc_pool.tile([64, 512], F32,
                                                      tag="rc", name=f"rc{bb}")
                                    _act_recip(nc, rc[:], l_cur[bb][0:64, :])
                                    st_rc[bb] = rc
                                    del l_cur[bb]
                        # stage C: pv + normalize broadcast + in-place store
                        j = i - LAG
                        if 0 <= j < NIT:
                            g, blk = iters[j]
                            b, r = divmod(j, 2)
                            pv_ps = ps_pv.tile([128, 512], F32, tag="pv")
                            nc.tensor.matmul(
                                pv_ps[:],
                                v_sb[:, blk, g * 128:(g + 1) * 128],
                                st_wT.pop(j)[:], start=True, stop=True)
                            bc_ps = ps_bc.tile([128, 512], F32, tag="bc")
                            nc.tensor.matmul(
                                bc_ps[:], ones_row_f[32 * r:32 * r + 1, :],
                                st_rc[b][32 * r:32 * r + 1, :],
                                start=True, stop=True)
                            if r == 1:
                                del st_rc[b]
                            bcb = bcb_pool.tile([128, 512], BF16, tag="bcb")
                            nc.vector.tensor_copy(bcb[:], bc_ps[:])
                            nc.vector.tensor_mul(
                                qT[:, g * G:(g + 1) * G,
                                   blk * 128:(blk + 1) * 128],
                                pv_ps[:], bcb[:])

                    chain = 0
                    for m in range(MQ):
                        wsb = wqk_pool.tile([128, KO, 128], BF16, tag="w")
                        nc.sync.dma_start(wsb[:], wq_t[m])
                        for h in range(2):
                            ps = psqk.tile([128, 512], F32, tag="ps")
                            for ko in range(KO):
                                nc.tensor.matmul(
                                    ps[:], wsb[:, ko, :],
                                    xt[:, ko, h * 512:(h + 1) * 512],
                                    start=(ko == 0), stop=(ko == KO - 1))
                            rope_evict(ps, qT[:, m, h * 512:(h + 1) * 512],
                                       h, m, bq_sb if use_bias else None)
                            chain += 1
                            if chain >= 10:
                                attn_step(chain - 10)
                    # tail: group 7's attention alone is ACT-bound (~50%
                    # PE duty), which lets HAM re-throttle and the out
                    # projection then starts at half clock. Weave the first
                    # out-projection chains (n=0, token blocks 0-1, using
                    # the now-idle psqk banks) into the tail steps as real
                    # PE work: the tail stays dense and out-proj effectively
                    # starts during it. Blocks 0-1 of n=0 are then skipped
                    # in the main out-proj loop.
                    pssA = [psqk.tile([128, 512], F32, tag="ps",
                                      name=f"psA{t}") for t in range(2)]
                    hk_done = 0
                    tail = list(range(chain - 9, NIT + LAG))
                    for idx, i in enumerate(tail):
                        attn_step(i)
                        target = min(MQ, (idx + 1) * 3)
                        if idx == len(tail) - 1:
                            target = MQ
                        while hk_done < target:
                            hk = hk_done
                            wo_sb = wmov_pool.tile([128, 512], BF16,
                                                   tag="wv")
                            nc.sync.dma_start(wo_sb[:], wo_t[hk, 0])
                            for t in range(2):
                                nc.tensor.matmul(
                                    pssA[t][:],
                                    qT[:, hk, t * 128:(t + 1) * 128],
                                    wo_sb[:], start=(hk == 0),
                                    stop=(hk == MQ - 1 and not use_bias))
                            hk_done += 1
                    if use_bias:
                        for t in range(2):
                            nc.tensor.matmul(pssA[t][:], ones_row[:],
                                             bo_sb[:, 0, :],
                                             start=False, stop=True)
                    for t in range(2):
                        oeA = bcb_pool.tile([128, 512], BF16, tag="bcb",
                                            name=f"oeA{t}")
                        nc.scalar.copy(oeA[:, 0:256], pssA[t][:, 0:256])
                        nc.vector.tensor_copy(oeA[:, 256:512],
                                              pssA[t][:, 256:512])
                        nc.sync.dma_start(
                            out[t * 128:(t + 1) * 128, 0:512], oeA[:])

            # ================= out projection =================
            with (
                tc.tile_pool(name="wop", bufs=8) as wo_pool,
                tc.tile_pool(name="oe", bufs=6) as oe_pool,
                tc.tile_pool(name="psO", bufs=8, space="PSUM") as psO,
            ):
                for n in range(NE):
                    mts = list(range(2, NBLK)) if n == 0 else list(range(NBLK))
                    pss = {mt: psO.tile([128, 512], F32, tag="ps",
                                        name=f"pso{n}_{mt}")
                           for mt in mts}
                    for hk in range(MQ):
                        wo_sb = wo_pool.tile([128, 512], BF16, tag="wo")
                        nc.sync.dma_start(wo_sb[:], wo_t[hk, n])
                        for mt in mts:
                            nc.tensor.matmul(
                                pss[mt][:],
                                qT[:, hk, mt * 128:(mt + 1) * 128],
                                wo_sb[:], start=(hk == 0),
                                stop=(hk == MQ - 1 and not use_bias))
                    if use_bias:
                        for mt in mts:
                            nc.tensor.matmul(pss[mt][:], ones_row[:],
                                             bo_sb[:, n, :],
                                             start=False, stop=True)
                    for mt in mts:
                        oe = oe_pool.tile([128, 512], BF16, tag="oe")
                        nc.scalar.copy(oe[:, 0:256], pss[mt][:, 0:256])
                        nc.vector.tensor_copy(oe[:, 256:512],
                                              pss[mt][:, 256:512])
                        nc.sync.dma_start(
                            out[mt * 128:(mt + 1) * 128,
                                n * 512:(n + 1) * 512], oe[:])

    return _split_excess_waits(nc)


_NC_CACHE = {}


def _get_nc(use_bias: bool):
    if use_bias not in _NC_CACHE:
        _NC_CACHE[use_bias] = _build(use_bias)
    return _NC_CACHE[use_bias]


def _prepare(x, wq, bq, wk, bk, wv, bv, wo, bo, mask):
    x = np.asarray(x, np.float32)
    wq = np.asarray(wq, np.float32)
    wk = np.asarray(wk, np.float32)
    wv = np.asarray(wv, np.float32)
    wo = np.asarray(wo, np.float32)
    bq = np.asarray(bq, np.float32)
    bk = np.asarray(bk, np.float32)
    bv = np.asarray(bv, np.float32)
    bo = np.asarray(bo, np.float32)
    mask = np.asarray(mask)

    use_bias = bool(bq.any() or bk.any() or bv.any() or bo.any())

    # weight layouts (shared across cores)
    wq_t = np.ascontiguousarray(
        wq.reshape(KO, 128, MQ, 128).transpose(2, 1, 0, 3)).astype(NPBF16)
    wk_t = np.ascontiguousarray(
        wk.reshape(KO, 128, MK, 128).transpose(2, 1, 0, 3)).astype(NPBF16)
    wv_t = np.ascontiguousarray(
        wv.reshape(KO, 128, 2, 512).transpose(0, 2, 1, 3)).astype(NPBF16)
    wo_t = np.ascontiguousarray(
        wo.reshape(MQ, 128, NE, 512).transpose(0, 2, 1, 3)).astype(NPBF16)

    # RoPE tables (positions are global sequence positions)
    inv = 1.0 / (ROPE_BASE ** (np.arange(0, D, 2, dtype=np.float32) / D))
    pos = np.arange(S, dtype=np.float32)
    ang = pos[:, None] * inv[None, :]                      # [S, 64]
    cos_full = np.concatenate([np.cos(ang), np.cos(ang)], -1).T  # [128, S]
    sin_half = np.sin(ang).T                               # [64, S]
    sin_eff = np.concatenate([-sin_half, sin_half], 0)     # [128, S]

    shards_per_b = NCORES // B                             # 4
    in_maps = []
    for c in range(NCORES):
        b = c // shards_per_b
        s0 = (c % shards_per_b) * TOK
        xs = x[b, s0:s0 + TOK]                             # [TOK, E]
        xT_t = np.ascontiguousarray(xs.T).astype(NPBF16).reshape(KO, 128, TOK)
        mshard = mask[b, s0:s0 + TOK].reshape(NBLK, BS)
        mb = np.where(mshard, np.float32(0.0), np.float32(-80.0)).astype(np.float32)
        im = {
            "xT": xT_t,
            "wq_t": wq_t, "wk_t": wk_t, "wv_t": wv_t, "wo_t": wo_t,
            "cos_t": np.ascontiguousarray(cos_full[:, s0:s0 + TOK]).astype(NPBF16),
            "sin_t": np.ascontiguousarray(sin_eff[:, s0:s0 + TOK]).astype(NPBF16),
            "mb_t": mb,
        }
        if use_bias:
            im["bq_t"] = bq.reshape(MQ, 128).copy()
            im["bk_t"] = bk.reshape(MK, 128).copy()
            im["bv_t"] = bv.reshape(2, 512).astype(NPBF16)
            im["bo_t"] = bo.reshape(NE, 512).astype(NPBF16)
        in_maps.append(im)

    return in_maps, use_bias


def _assemble(results):
    shards_per_b = NCORES // B
    out = np.empty((B, S, E), np.float32)
    for c in range(NCORES):
        b = c // shards_per_b
        s0 = (c % shards_per_b) * TOK
        out[b, s0:s0 + TOK] = np.asarray(results[c]["out"], np.float32)
    return out


def kernel(**inputs):
    in_maps, use_bias = _prepare(**inputs)
    nc = _get_nc(use_bias)
    res = run_bass_kernel_spmd(nc, in_maps, core_ids=list(range(NCORES)))
    return _assemble(res.results)


# revision 19
# speedup vs baseline: 1.0246x; 1.0246x over previous
"""Trainium2 Bass kernel for CosmicMultiHeadAttention (block-local flash attention).

Sharding: the 8192 tokens (B=2 x S=4096) are split into 8 shards of 1024
tokens (batch-major). Attention is block-local with 128-token blocks, so
1024-token shards (8 blocks each) have zero cross-shard dependencies: every
core runs the full layer (QKV proj + RoPE + block attention + out proj) for
its own tokens. No collectives.

v2 layout strategy (single pass over all 1024 tokens, weights loaded once):
  - x is transposed on the host to xT [E, tok] so the E contraction sits on
    the partition axis; resident in SBUF for the whole projection phase.
  - V proj first: its ko-loop matmuls are paced by the arriving xt DMAs, so
    the PE starts ~3us in and stays busy through the initial load.
    Swapped operands (lhsT = xt tiles) land v natural [tok, hd] - exactly
    the PV-matmul lhsT layout.
  - K/Q projections: lhsT = weight tiles (1MB DMA each, loaded ONCE),
    rhs = xt -> psum kT/qT [hd, tok]. RoPE applied during psum eviction
    (ACT copy + partition-shift DMA + DVE).
  - attention: 64 iterations (8 blocks x 8 kv-heads, 4 grouped q-heads,
    N=512), 3-stage software pipeline across engines:
      stage1(i):   s = kT.T @ qT;  ACT exp (scale, mask-bias) -> wT (SBUF)
      stage2(i-1): l = ones.T @ wT; DVE recip; pv = v.T @ wT
      stage3(i-2): bc = broadcast(1/l) via K=1 matmul; DVE mul -> oT
    PE order per step: s(i), l(i-1), pv(i-1), bc(i-2) - every operand was
    produced >= 1 full step (~850ns) earlier, so the PE never waits on the
    ACT-exp / DVE-recip chains and HAM stays at K=8/8.
  - out projection: lhsT = oT tiles (on-chip), rhs = wo tiles (loaded once),
    8 psum banks accumulate over the 32 head k-tiles, evictions alternate
    ACT/DVE so the next n-tile's first matmuls aren't serialized behind a
    single engine.
"""

import sys

if '/opt/trn_rl_repo' not in sys.path:
    sys.path.insert(0, '/opt/trn_rl_repo')

import numpy as np
import ml_dtypes

import concourse.bass as bass
import concourse.tile as tile
from concourse import mybir
from concourse.bass_utils import run_bass_kernel_spmd

BF16 = mybir.dt.bfloat16
F32 = mybir.dt.float32
NPBF16 = ml_dtypes.bfloat16

B, S, E = 2, 4096, 4096
HQ, HKV, D = 32, 8, 128
BS = 128
ROPE_BASE = 10000.0
NCORES = 8
TOK = (B * S) // NCORES          # 1024 tokens per core
KO = E // 128                    # 32 k-tiles over E
MQ = (HQ * D) // 128             # 32 q head-tiles
MK = (HKV * D) // 128            # 8 k head-tiles
G = HQ // HKV                    # 4 q heads per kv head
NBLK = TOK // BS                 # 8 blocks per core
NE = E // 512                    # 8 out-proj column tiles
SCALE = 1.0 / float(np.sqrt(D))


# ---------------------------------------------------------------------------
# The walrus build in this image rejects instructions carrying more than one
# "sem-ge" sync wait ("Too many sync wait commands"; Drain/CTRL accepts
# none). Tile's scheduler freely attaches several. Post-pass: keep at most
# one ge-wait per instruction (none on Drain) and move the excess onto
# EventSemaphore carrier instructions inserted immediately before, on the
# same engine - program order preserves the blocking semantics exactly.
# ---------------------------------------------------------------------------
def _split_excess_waits(nc):
    import bass_rust
    ctr = 0
    for f in nc.m.functions:
        for bb in f.blocks:
            out_list = []
            for inst in bb.instructions:
                si = inst.sync_info
                all_waits = list(si.on_wait) if si and si.on_wait else []
                ge = [w for w in all_waits if 'ge' in w.wait_mode]
                eq = [w for w in all_waits if 'ge' not in w.wait_mode]
                keep_n = 0 if type(inst).__name__ == 'InstDrain' else 1
                if len(ge) > keep_n:
                    extra, keep = ge[:-keep_n] if keep_n else ge, \
                        ge[-keep_n:] if keep_n else []
                    for w in extra:
                        ctr += 1
                        es = mybir.InstEventSemaphore(
                            name=f'waitsplit_{ctr}', engine=inst.engine,
                            ins=[], outs=[],
                            sync_info=bass_rust.SyncInfo(
                                on_wait=[w], on_update=[]))
                        out_list.append(es)
                    si.on_wait = eq + keep
                out_list.append(inst)
            bb.instructions[:] = out_list
    return nc


def _act_recip(nc, out_ap, in_ap):
    # ACT LUT reciprocal, emitted directly: the bass wrapper refuses
    # Reciprocal on accuracy grounds, but ~1e-3 relative is ample here and
    # every DVE reciprocal path is either ~6 passes (3.3us per call) or
    # unsupported by this walrus build.
    se = nc.scalar
    ins = [se.lower_ap(in_ap)]
    for arg in (0.0, 1.0, 0.0):  # bias, scale, alpha
        ins.append(mybir.ImmediateValue(dtype=mybir.dt.float32, value=arg))
    return se.add_instruction(
        mybir.InstActivation(
            name=se.bass.get_next_instruction_name(),
            func=mybir.ActivationFunctionType.Reciprocal,
            ins=ins, outs=[se.lower_ap(out_ap)]))


def _build(use_bias: bool):
    nc = bass.Bass()

    xT = nc.dram_tensor("xT", [KO, 128, TOK], BF16, kind="ExternalInput")
    wq_t = nc.dram_tensor("wq_t", [MQ, 128, KO, 128], BF16, kind="ExternalInput")
    wk_t = nc.dram_tensor("wk_t", [MK, 128, KO, 128], BF16, kind="ExternalInput")
    wv_t = nc.dram_tensor("wv_t", [KO, 2, 128, 512], BF16, kind="ExternalInput")
    wo_t = nc.dram_tensor("wo_t", [MQ, NE, 128, 512], BF16, kind="ExternalInput")
    cos_t = nc.dram_tensor("cos_t", [128, TOK], BF16, kind="ExternalInput")
    sin_t = nc.dram_tensor("sin_t", [128, TOK], BF16, kind="ExternalInput")
    mb_t = nc.dram_tensor("mb_t", [NBLK, 128], F32, kind="ExternalInput")
    if use_bias:
        bq_t = nc.dram_tensor("bq_t", [MQ, 128], F32, kind="ExternalInput")
        bk_t = nc.dram_tensor("bk_t", [MK, 128], F32, kind="ExternalInput")
        bv_t = nc.dram_tensor("bv_t", [2, 512], BF16, kind="ExternalInput")
        bo_t = nc.dram_tensor("bo_t", [NE, 512], BF16, kind="ExternalInput")
    out = nc.dram_tensor("out", [TOK, E], BF16, kind="ExternalOutput")

    with tile.TileContext(nc) as tc:
        with (
            tc.tile_pool(name="const", bufs=1) as cpool,
            tc.tile_pool(name="keep", bufs=1) as keep,
        ):
            # ---- constants (DMAs issued after xt below; xt is critical path) ----
            cos_sb = cpool.tile([128, TOK], BF16, tag="cos")
            sin_sb = cpool.tile([128, TOK], BF16, tag="sin")
            mb_sb = cpool.tile([128, NBLK], F32, tag="mb")
            ones_col = cpool.tile([128, 1], BF16, tag="ones_col")
            nc.vector.memset(ones_col[:], 1.0)
            ones_row_f = cpool.tile([64, 128], F32, tag="ones_row_f")
            nc.vector.memset(ones_row_f[:], 1.0)
            if use_bias:
                ones_row = cpool.tile([1, 128], BF16, tag="ones_row")
                nc.vector.memset(ones_row[:], 1.0)
                bq_sb = cpool.tile([128, MQ], F32, tag="bq")
                nc.sync.dma_start(bq_sb[:], bq_t.rearrange("m p -> p m"))
                bk_sb = cpool.tile([128, MK], F32, tag="bk")
                nc.sync.dma_start(bk_sb[:], bk_t.rearrange("m p -> p m"))
                bv_sb = cpool.tile([1, 2, 512], BF16, tag="bv")
                nc.sync.dma_start(bv_sb[:], bv_t[None, :, :])
                bo_sb = cpool.tile([1, NE, 512], BF16, tag="bo")
                nc.sync.dma_start(bo_sb[:], bo_t[None, :, :])

            # persistent intermediates
            qT = keep.tile([128, MQ, TOK], BF16, tag="qT")
            kT = keep.tile([128, MK, TOK], BF16, tag="kT")
            v_sb = keep.tile([128, NBLK, HKV * D], BF16, tag="v")

            # ================= projection phase =================
            with (
                tc.tile_pool(name="xtp", bufs=1) as xt_pool,
                tc.tile_pool(name="wqk", bufs=2) as wqk_pool,
                tc.tile_pool(name="wmov", bufs=4) as wmov_pool,
                tc.tile_pool(name="rope", bufs=2) as rope_pool,
            ):
                # pre-issue the first wv tiles, then interleave the xt
                # slices: V-proj's first matmuls need (xt[0], wv[0]) - if all
                # 32 xt DMAs queue first, wv[0] lands ~30us late and the PE
                # sits idle exactly that long.
                xt = xt_pool.tile([128, KO, TOK], BF16, tag="xt")
                wv_pre = []
                for k in range(4):
                    wv_sb = wmov_pool.tile([128, 512], BF16, tag="wv",
                                           name=f"wvp{k}")
                    nc.sync.dma_start(wv_sb[:], wv_t[k, 0])
                    wv_pre.append(wv_sb)
                for ko in range(KO):
                    nc.sync.dma_start(xt[:, ko, :], xT[ko])
                nc.sync.dma_start(cos_sb[:], cos_t[:, :])
                nc.sync.dma_start(sin_sb[:], sin_t[:, :])
                nc.sync.dma_start(mb_sb[:], mb_t.rearrange("b p -> p b"))

                # ---- V projection (paced by xt arrival) ----
                with tc.tile_pool(name="psv", bufs=8, space="PSUM") as psv:
                    for n in range(2):
                        pss = [psv.tile([128, 512], F32, tag="ps",
                                        name=f"psv{n}_{mt}")
                               for mt in range(NBLK)]
                        for ko in range(KO):
                            if n == 0 and ko < 4:
                                wv_sb = wv_pre[ko]
                            else:
                                wv_sb = wmov_pool.tile([128, 512], BF16,
                                                       tag="wv")
                                nc.sync.dma_start(wv_sb[:], wv_t[ko, n])
                            for mt in range(NBLK):
                                nc.tensor.matmul(
                                    pss[mt][:],
                                    xt[:, ko, mt * 128:(mt + 1) * 128],
                                    wv_sb[:], start=(ko == 0),
                                    stop=(ko == KO - 1 and not use_bias))
                        if use_bias:
                            for mt in range(NBLK):
                                nc.tensor.matmul(pss[mt][:], ones_row[:],
                                                 bv_sb[:, n, :],
                                                 start=False, stop=True)
                        for mt in range(NBLK):
                            dst = v_sb[:, mt, n * 512:(n + 1) * 512]
                            nc.scalar.copy(dst[:, 0:256], pss[mt][:, 0:256])
                            nc.vector.tensor_copy(dst[:, 256:512],
                                                  pss[mt][:, 256:512])

                # ---- K/Q projections with fused RoPE eviction ----
                def rope_evict(ps, dst_ap, h, m, bias_sb):
                    ts = slice(h * 512, (h + 1) * 512)
                    qa = rope_pool.tile([128, 512], BF16, tag="qa")
                    if bias_sb is not None:
                        nc.scalar.add(qa[:], ps[:], bias_sb[:, m:m + 1])
                    else:
                        nc.scalar.copy(qa[:], ps[:])
                    qsh = rope_pool.tile([128, 512], BF16, tag="qsh")
                    nc.sync.dma_start(qsh[0:64, :], qa[64:128, :])
                    nc.sync.dma_start(qsh[64:128, :], qa[0:64, :])
                    t1 = rope_pool.tile([128, 512], BF16, tag="t1")
                    nc.vector.tensor_mul(t1[:], qa[:], cos_sb[:, ts])
                    t2 = rope_pool.tile([128, 512], BF16, tag="t2")
                    nc.vector.tensor_mul(t2[:], qsh[:], sin_sb[:, ts])
                    nc.vector.tensor_add(dst_ap, t1[:], t2[:])

                # ---- K proj, then Q proj with woven attention ----
                # The attention pipeline rides inside the Q-projection
                # stream: one attention step is emitted after each Q psum
                # chain (64 chains <-> 64 steps, offset 9 so group g's
                # attention runs while group g+1 projects). Every attention
                # operand is then ~7us old when the PE reaches it, and the
                # ACT Exp<->Reciprocal table reloads (1.3us each) hide
                # entirely behind projection matmuls. Attention output is
                # normalized in-place into qT (each slice is dead as a query
                # once its s-matmul has run), which keeps SBUF under budget.
                with (
                    tc.tile_pool(name="psqk", bufs=2, space="PSUM") as psqk,
                    tc.tile_pool(name="ps_s", bufs=2, space="PSUM") as ps_s,
                    tc.tile_pool(name="ps_l", bufs=2, space="PSUM") as ps_l,
                    tc.tile_pool(name="ps_pv", bufs=1, space="PSUM") as ps_pv,
                    tc.tile_pool(name="ps_bc", bufs=1, space="PSUM") as ps_bc,
                    tc.tile_pool(name="attn", bufs=6) as attn_pool,
                    tc.tile_pool(name="bcbp", bufs=2) as bcb_pool,
                    tc.tile_pool(name="rcp", bufs=3) as rc_pool,
                ):
                    for m in range(MK):
                        wsb = wqk_pool.tile([128, KO, 128], BF16, tag="w")
                        nc.sync.dma_start(wsb[:], wk_t[m])
                        for h in range(2):
                            ps = psqk.tile([128, 512], F32, tag="ps")
                            for ko in range(KO):
                                nc.tensor.matmul(
                                    ps[:], wsb[:, ko, :],
                                    xt[:, ko, h * 512:(h + 1) * 512],
                                    start=(ko == 0), stop=(ko == KO - 1))
                            rope_evict(ps, kT[:, m, h * 512:(h + 1) * 512],
                                       h, m, bk_sb if use_bias else None)

                    iters = [(g, blk) for g in range(HKV)
                             for blk in range(NBLK)]
                    NIT = len(iters)
                    LAG = 5
                    st_wT = {}
                    st_rc = {}
                    l_cur = {}

                    def attn_step(i):
                        # stage A: scores + exp for iteration i
                        if i < NIT:
                            g, blk = iters[i]
                            tq = slice(blk * 128, (blk + 1) * 128)
                            s_ps = ps_s.tile([128, 512], F32, tag="s")
                            nc.tensor.matmul(
                                s_ps[:], kT[:, g, tq],
                                qT[:, g * G:(g + 1) * G, tq],
                                start=True, stop=True)
                            wT = attn_pool.tile([128, 512], BF16, tag="wT")
                            nc.scalar.activation(
                                out=wT[:], in_=s_ps[:],
                                func=mybir.ActivationFunctionType.Exp,
                                scale=SCALE, bias=mb_sb[:, blk:blk + 1])
                            st_wT[i] = wT
                        # stage B: row-sum; adjacent reciprocal pair per 4
                        j = i - 1
                        if 0 <= j < NIT:
                            b, r = divmod(j, 2)
                            if r == 0:
                                l_cur[b] = ps_l.tile([128, 512], F32,
                                                     tag="l", name=f"l{b}")
                            nc.tensor.matmul(
                                l_cur[b][32 * r:32 * r + 1, :], ones_col[:],
                                st_wT[j][:], start=True, stop=True)
                            if r == 1 and (b % 2 == 1 or j == NIT - 1):
                                for bb in ([b - 1, b] if b % 2 == 1 else [b]):
                                    rc = rc_pool.tile([64, 512], F32,
                                                      tag="rc", name=f"rc{bb}")
                                    _act_recip(nc, rc[:], l_cur[bb][0:64, :])
                                    st_rc[bb] = rc
                                    del l_cur[bb]
                        # stage C: pv + normalize broadcast + in-place store
                        j = i - LAG
                        if 0 <= j < NIT:
                            g, blk = iters[j]
                            b, r = divmod(j, 2)
                            pv_ps = ps_pv.tile([128, 512], F32, tag="pv")
                            nc.tensor.matmul(
                                pv_ps[:],
                                v_sb[:, blk, g * 128:(g + 1) * 128],
                                st_wT.pop(j)[:], start=True, stop=True)
                            bc_ps = ps_bc.tile([128, 512], F32, tag="bc")
                            nc.tensor.matmul(
                                bc_ps[:], ones_row_f[32 * r:32 * r + 1, :],
                                st_rc[b][32 * r:32 * r + 1, :],
                                start=True, stop=True)
                            if r == 1:
                                del st_rc[b]
                            bcb = bcb_pool.tile([128, 512], BF16, tag="bcb")
                            nc.vector.tensor_copy(bcb[:], bc_ps[:])
                            nc.vector.tensor_mul(
                                qT[:, g * G:(g + 1) * G,
                                   blk * 128:(blk + 1) * 128],
                                pv_ps[:], bcb[:])

                    chain = 0
                    for m in range(MQ):
                        wsb = wqk_pool.tile([128, KO, 128], BF16, tag="w")
                        nc.sync.dma_start(wsb[:], wq_t[m])
                        for h in range(2):
                            ps = psqk.tile([128, 512], F32, tag="ps")
                            for ko in range(KO):
                                nc.tensor.matmul(
                                    ps[:], wsb[:, ko, :],
                                    xt[:, ko, h * 512:(h + 1) * 512],
                                    start=(ko == 0), stop=(ko == KO - 1))
                            rope_evict(ps, qT[:, m, h * 512:(h + 1) * 512],
                                       h, m, bq_sb if use_bias else None)
                            chain += 1
                            if chain >= 10:
                                attn_step(chain - 10)
                    # tail: group 7's attention alone is ACT-bound (~50%
                    # PE duty), which lets HAM re-throttle and the out
                    # projection then starts at half clock. Weave the first
                    # out-projection chains (n=0, token blocks 0-1, using
                    # the now-idle psqk banks) into the tail steps as real
                    # PE work: the tail stays dense and out-proj effectively
                    # starts during it. Blocks 0-1 of n=0 are then skipped
                    # in the main out-proj loop.
                    pssA = [psqk.tile([128, 512], F32, tag="ps",
                                      name=f"psA{t}") for t in range(2)]
                    hk_done = 0
                    tail = list(range(chain - 9, NIT + LAG))
                    for idx, i in enumerate(tail):
                        attn_step(i)
                        target = min(MQ, (idx + 1) * 3)
                        if idx == len(tail) - 1:
                            target = MQ
                        while hk_done < target:
                            hk = hk_done
                            wo_sb = wmov_pool.tile([128, 512], BF16,
                                                   tag="wv")
                            nc.sync.dma_start(wo_sb[:], wo_t[hk, 0])
                            for t in range(2):
                                nc.tensor.matmul(
                                    pssA[t][:],
                                    qT[:, hk, t * 128:(t + 1) * 128],
                                    wo_sb[:], start=(hk == 0),
                                    stop=(hk == MQ - 1 and not use_bias))
                            hk_done += 1
                    if use_bias:
                        for t in range(2):
                            nc.tensor.matmul(pssA[t][:], ones_row[:],
                                             bo_sb[:, 0, :],
                                             start=False, stop=True)
                    for t in range(2):
                        oeA = bcb_pool.tile([128, 512], BF16, tag="bcb",
                                            name=f"oeA{t}")
                        nc.scalar.copy(oeA[:, 0:256], pssA[t][:, 0:256])
                        nc.vector.tensor_copy(oeA[:, 256:512],
                                              pssA[t][:, 256:512])
                        nc.sync.dma_start(
                            out[t * 128:(t + 1) * 128, 0:512], oeA[:])

            # ================= out projection =================
            with (
                tc.tile_pool(name="wop", bufs=8) as wo_pool,
                tc.tile_pool(name="oe", bufs=6) as oe_pool,
                tc.tile_pool(name="psO", bufs=8, space="PSUM") as psO,
            ):
                for n in range(NE):
                    mts = list(range(2, NBLK)) if n == 0 else list(range(NBLK))
                    pss = {mt: psO.tile([128, 512], F32, tag="ps",
                                        name=f"pso{n}_{mt}")
                           for mt in mts}
                    for hk in range(MQ):
                        wo_sb = wo_pool.tile([128, 512], BF16, tag="wo")
                        nc.sync.dma_start(wo_sb[:], wo_t[hk, n])
                        for mt in mts:
                            nc.tensor.matmul(
                                pss[mt][:],
                                qT[:, hk, mt * 128:(mt + 1) * 128],
                                wo_sb[:], start=(hk == 0),
                                stop=(hk == MQ - 1 and not use_bias))
                    if use_bias:
                        for mt in mts:
                            nc.tensor.matmul(pss[mt][:], ones_row[:],
                                             bo_sb[:, n, :],
                                             start=False, stop=True)
                    for mt in mts:
                        oe = oe_pool.tile([128, 512], BF16, tag="oe")
                        nc.scalar.copy(oe[:, 0:256], pss[mt][:, 0:256])
                        nc.vector.tensor_copy(oe[:, 256:512],
                                              pss[mt][:, 256:512])
                        nc.sync.dma_start(
                            out[mt * 128:(mt + 1) * 128,
                                n * 512:(n + 1) * 512], oe[:])

    return _split_excess_waits(nc)


_NC_CACHE = {}


def _get_nc(use_bias: bool):
    if use_bias not in _NC_CACHE:
        _NC_CACHE[use_bias] = _build(use_bias)
    return _NC_CACHE[use_bias]


def _prepare(x, wq, bq, wk, bk, wv, bv, wo, bo, mask):
    x = np.asarray(x, np.float32)
    wq = np.asarray(wq, np.float32)
    wk = np.asarray(wk, np.float32)
    wv = np.asarray(wv, np.float32)
    wo = np.asarray(wo, np.float32)
    bq = np.asarray(bq, np.float32)
    bk = np.asarray(bk, np.float32)
    bv = np.asarray(bv, np.float32)
    bo = np.asarray(bo, np.float32)
    mask = np.asarray(mask)

    use_bias = bool(bq.any() or bk.any() or bv.any() or bo.any())

    # weight layouts (shared across cores)
    wq_t = np.ascontiguousarray(
        wq.reshape(KO, 128, MQ, 128).transpose(2, 1, 0, 3)).astype(NPBF16)
    wk_t = np.ascontiguousarray(
        wk.reshape(KO, 128, MK, 128).transpose(2, 1, 0, 3)).astype(NPBF16)
    wv_t = np.ascontiguousarray(
        wv.reshape(KO, 128, 2, 512).transpose(0, 2, 1, 3)).astype(NPBF16)
    wo_t = np.ascontiguousarray(
        wo.reshape(MQ, 128, NE, 512).transpose(0, 2, 1, 3)).astype(NPBF16)

    # RoPE tables (positions are global sequence positions)
    inv = 1.0 / (ROPE_BASE ** (np.arange(0, D, 2, dtype=np.float32) / D))
    pos = np.arange(S, dtype=np.float32)
    ang = pos[:, None] * inv[None, :]                      # [S, 64]
    cos_full = np.concatenate([np.cos(ang), np.cos(ang)], -1).T  # [128, S]
    sin_half = np.sin(ang).T                               # [64, S]
    sin_eff = np.concatenate([-sin_half, sin_half], 0)     # [128, S]

    shards_per_b = NCORES // B                             # 4
    in_maps = []
    for c in range(NCORES):
        b = c // shards_per_b
        s0 = (c % shards_per_b) * TOK
        xs = x[b, s0:s0 + TOK]                             # [TOK, E]
        xT_t = np.ascontiguousarray(xs.T).astype(NPBF16).reshape(KO, 128, TOK)
        mshard = mask[b, s0:s0 + TOK].reshape(NBLK, BS)
        mb = np.where(mshard, np.float32(0.0), np.float32(-80.0)).astype(np.float32)
        im = {
            "xT": xT_t,
            "wq_t": wq_t, "wk_t": wk_t, "wv_t": wv_t, "wo_t": wo_t,
            "cos_t": np.ascontiguousarray(cos_full[:, s0:s0 + TOK]).astype(NPBF16),
            "sin_t": np.ascontiguousarray(sin_eff[:, s0:s0 + TOK]).astype(NPBF16),
            "mb_t": mb,
        }
        if use_bias:
            im["bq_t"] = bq.reshape(MQ, 128).copy()
            im["bk_t"] = bk.reshape(MK, 128).copy()
            im["bv_t"] = bv.reshape(2, 512).astype(NPBF16)
            im["bo_t"] = bo.reshape(NE, 512).astype(NPBF16)
        in_maps.append(im)

    return in_maps, use_bias


def _assemble(results):
    shards_per_b = NCORES // B
    out = np.empty((B, S, E), np.float32)
    for c in range(NCORES):
        b = c // shards_per_b
        s0 = (c % shards_per_b) * TOK
        out[b, s0:s0 + TOK] = np.asarray(results[c]["out"], np.float32)
    return out


def kernel(**inputs):
    in_maps, use_bias = _prepare(**inputs)
    nc = _get_nc(use_bias)
    res = run_bass_kernel_spmd(nc, in_maps, core_ids=list(range(NCORES)))
    return _assemble(res.results)


# revision 21
# speedup vs baseline: 1.0323x; 1.0076x over previous
"""Trainium2 Bass kernel for CosmicMultiHeadAttention (block-local flash attention).

Sharding: the 8192 tokens (B=2 x S=4096) are split into 8 shards of 1024
tokens (batch-major). Attention is block-local with 128-token blocks, so
1024-token shards (8 blocks each) have zero cross-shard dependencies: every
core runs the full layer (QKV proj + RoPE + block attention + out proj) for
its own tokens. No collectives.

v2 layout strategy (single pass over all 1024 tokens, weights loaded once):
  - x is transposed on the host to xT [E, tok] so the E contraction sits on
    the partition axis; resident in SBUF for the whole projection phase.
  - V proj first: its ko-loop matmuls are paced by the arriving xt DMAs, so
    the PE starts ~3us in and stays busy through the initial load.
    Swapped operands (lhsT = xt tiles) land v natural [tok, hd] - exactly
    the PV-matmul lhsT layout.
  - K/Q projections: lhsT = weight tiles (1MB DMA each, loaded ONCE),
    rhs = xt -> psum kT/qT [hd, tok]. RoPE applied during psum eviction
    (ACT copy + partition-shift DMA + DVE).
  - attention: 64 iterations (8 blocks x 8 kv-heads, 4 grouped q-heads,
    N=512), 3-stage software pipeline across engines:
      stage1(i):   s = kT.T @ qT;  ACT exp (scale, mask-bias) -> wT (SBUF)
      stage2(i-1): l = ones.T @ wT; DVE recip; pv = v.T @ wT
      stage3(i-2): bc = broadcast(1/l) via K=1 matmul; DVE mul -> oT
    PE order per step: s(i), l(i-1), pv(i-1), bc(i-2) - every operand was
    produced >= 1 full step (~850ns) earlier, so the PE never waits on the
    ACT-exp / DVE-recip chains and HAM stays at K=8/8.
  - out projection: lhsT = oT tiles (on-chip), rhs = wo tiles (loaded once),
    8 psum banks accumulate over the 32 head k-tiles, evictions alternate
    ACT/DVE so the next n-tile's first matmuls aren't serialized behind a
    single engine.
"""

import sys

if '/opt/trn_rl_repo' not in sys.path:
    sys.path.insert(0, '/opt/trn_rl_repo')

import numpy as np
import ml_dtypes

import concourse.bass as bass
import concourse.tile as tile
from concourse import mybir
from concourse.bass_utils import run_bass_kernel_spmd

BF16 = mybir.dt.bfloat16
F32 = mybir.dt.float32
NPBF16 = ml_dtypes.bfloat16

B, S, E = 2, 4096, 4096
HQ, HKV, D = 32, 8, 128
BS = 128
ROPE_BASE = 10000.0
NCORES = 8
TOK = (B * S) // NCORES          # 1024 tokens per core
KO = E // 128                    # 32 k-tiles over E
MQ = (HQ * D) // 128             # 32 q head-tiles
MK = (HKV * D) // 128            # 8 k head-tiles
G = HQ // HKV                    # 4 q heads per kv head
NBLK = TOK // BS                 # 8 blocks per core
NE = E // 512                    # 8 out-proj column tiles
SCALE = 1.0 / float(np.sqrt(D))


# ---------------------------------------------------------------------------
# The walrus build in this image rejects instructions carrying more than one
# "sem-ge" sync wait ("Too many sync wait commands"; Drain/CTRL accepts
# none). Tile's scheduler freely attaches several. Post-pass: keep at most
# one ge-wait per instruction (none on Drain) and move the excess onto
# EventSemaphore carrier instructions inserted immediately before, on the
# same engine - program order preserves the blocking semantics exactly.
# ---------------------------------------------------------------------------
def _split_excess_waits(nc):
    import bass_rust
    ctr = 0
    for f in nc.m.functions:
        for bb in f.blocks:
            out_list = []
            for inst in bb.instructions:
                si = inst.sync_info
                all_waits = list(si.on_wait) if si and si.on_wait else []
                ge = [w for w in all_waits if 'ge' in w.wait_mode]
                eq = [w for w in all_waits if 'ge' not in w.wait_mode]
                keep_n = 0 if type(inst).__name__ == 'InstDrain' else 1
                if len(ge) > keep_n:
                    extra, keep = ge[:-keep_n] if keep_n else ge, \
                        ge[-keep_n:] if keep_n else []
                    for w in extra:
                        ctr += 1
                        es = mybir.InstEventSemaphore(
                            name=f'waitsplit_{ctr}', engine=inst.engine,
                            ins=[], outs=[],
                            sync_info=bass_rust.SyncInfo(
                                on_wait=[w], on_update=[]))
                        out_list.append(es)
                    si.on_wait = eq + keep
                out_list.append(inst)
            bb.instructions[:] = out_list
    return nc


def _act_recip(nc, out_ap, in_ap):
    # ACT LUT reciprocal, emitted directly: the bass wrapper refuses
    # Reciprocal on accuracy grounds, but ~1e-3 relative is ample here and
    # every DVE reciprocal path is either ~6 passes (3.3us per call) or
    # unsupported by this walrus build.
    se = nc.scalar
    ins = [se.lower_ap(in_ap)]
    for arg in (0.0, 1.0, 0.0):  # bias, scale, alpha
        ins.append(mybir.ImmediateValue(dtype=mybir.dt.float32, value=arg))
    return se.add_instruction(
        mybir.InstActivation(
            name=se.bass.get_next_instruction_name(),
            func=mybir.ActivationFunctionType.Reciprocal,
            ins=ins, outs=[se.lower_ap(out_ap)]))


def _build(use_bias: bool):
    nc = bass.Bass()

    xT = nc.dram_tensor("xT", [KO, 128, TOK], BF16, kind="ExternalInput")
    wq_t = nc.dram_tensor("wq_t", [MQ, 128, KO, 128], BF16, kind="ExternalInput")
    wk_t = nc.dram_tensor("wk_t", [MK, 128, KO, 128], BF16, kind="ExternalInput")
    wv_t = nc.dram_tensor("wv_t", [KO // 2, 2, 128, 2, 512], BF16, kind="ExternalInput")
    wo_t = nc.dram_tensor("wo_t", [MQ // 2, NE, 128, 2, 512], BF16, kind="ExternalInput")
    cos_t = nc.dram_tensor("cos_t", [128, TOK], BF16, kind="ExternalInput")
    sin_t = nc.dram_tensor("sin_t", [128, TOK], BF16, kind="ExternalInput")
    mb_t = nc.dram_tensor("mb_t", [NBLK, 128], F32, kind="ExternalInput")
    if use_bias:
        bq_t = nc.dram_tensor("bq_t", [MQ, 128], F32, kind="ExternalInput")
        bk_t = nc.dram_tensor("bk_t", [MK, 128], F32, kind="ExternalInput")
        bv_t = nc.dram_tensor("bv_t", [2, 512], BF16, kind="ExternalInput")
        bo_t = nc.dram_tensor("bo_t", [NE, 512], BF16, kind="ExternalInput")
    out = nc.dram_tensor("out", [TOK, E], BF16, kind="ExternalOutput")

    with tile.TileContext(nc) as tc:
        with (
            tc.tile_pool(name="const", bufs=1) as cpool,
            tc.tile_pool(name="keep", bufs=1) as keep,
        ):
            # ---- constants (DMAs issued after xt below; xt is critical path) ----
            cos_sb = cpool.tile([128, TOK], BF16, tag="cos")
            sin_sb = cpool.tile([128, TOK], BF16, tag="sin")
            mb_sb = cpool.tile([128, NBLK], F32, tag="mb")
            ones_col = cpool.tile([128, 1], BF16, tag="ones_col")
            nc.vector.memset(ones_col[:], 1.0)
            ones_row_f = cpool.tile([64, 128], F32, tag="ones_row_f")
            nc.vector.memset(ones_row_f[:], 1.0)
            if use_bias:
                ones_row = cpool.tile([1, 128], BF16, tag="ones_row")
                nc.vector.memset(ones_row[:], 1.0)
                bq_sb = cpool.tile([128, MQ], F32, tag="bq")
                nc.sync.dma_start(bq_sb[:], bq_t.rearrange("m p -> p m"))
                bk_sb = cpool.tile([128, MK], F32, tag="bk")
                nc.sync.dma_start(bk_sb[:], bk_t.rearrange("m p -> p m"))
                bv_sb = cpool.tile([1, 2, 512], BF16, tag="bv")
                nc.sync.dma_start(bv_sb[:], bv_t[None, :, :])
                bo_sb = cpool.tile([1, NE, 512], BF16, tag="bo")
                nc.sync.dma_start(bo_sb[:], bo_t[None, :, :])

            # persistent intermediates
            qT = keep.tile([128, MQ, TOK], BF16, tag="qT")
            kT = keep.tile([128, MK, TOK], BF16, tag="kT")
            v_sb = keep.tile([128, NBLK, HKV * D], BF16, tag="v")

            # ================= projection phase =================
            with (
                tc.tile_pool(name="xtp", bufs=1) as xt_pool,
                tc.tile_pool(name="wqk", bufs=2) as wqk_pool,
                tc.tile_pool(name="wmov", bufs=2) as wmov_pool,
                tc.tile_pool(name="rope", bufs=2) as rope_pool,
            ):
                # pre-issue the first wv tiles, then interleave the xt
                # slices: V-proj's first matmuls need (xt[0], wv[0]) - if all
                # 32 xt DMAs queue first, wv[0] lands ~30us late and the PE
                # sits idle exactly that long.
                xt = xt_pool.tile([128, KO, TOK], BF16, tag="xt")
                wv_pre = []
                for k in range(2):
                    wv_sb = wmov_pool.tile([128, 2, 512], BF16, tag="wv",
                                           name=f"wvp{k}")
                    nc.sync.dma_start(wv_sb[:], wv_t[k, 0])
                    wv_pre.append(wv_sb)
                for ko in range(KO):
                    nc.sync.dma_start(xt[:, ko, :], xT[ko])
                nc.sync.dma_start(cos_sb[:], cos_t[:, :])
                nc.sync.dma_start(sin_sb[:], sin_t[:, :])
                nc.sync.dma_start(mb_sb[:], mb_t.rearrange("b p -> p b"))

                # ---- V projection (paced by xt arrival) ----
                with tc.tile_pool(name="psv", bufs=8, space="PSUM") as psv:
                    for n in range(2):
                        pss = [psv.tile([128, 512], F32, tag="ps",
                                        name=f"psv{n}_{mt}")
                               for mt in range(NBLK)]
                        for kp in range(KO // 2):
                            if n == 0 and kp < 2:
                                wv_sb = wv_pre[kp]
                            else:
                                wv_sb = wmov_pool.tile([128, 2, 512], BF16,
                                                       tag="wv")
                                nc.sync.dma_start(wv_sb[:], wv_t[kp, n])
                            for k2 in range(2):
                                ko = 2 * kp + k2
                                for mt in range(NBLK):
                                    nc.tensor.matmul(
                                        pss[mt][:],
                                        xt[:, ko, mt * 128:(mt + 1) * 128],
                                        wv_sb[:, k2, :], start=(ko == 0),
                                        stop=(ko == KO - 1 and not use_bias))
                        if use_bias:
                            for mt in range(NBLK):
                                nc.tensor.matmul(pss[mt][:], ones_row[:],
                                                 bv_sb[:, n, :],
                                                 start=False, stop=True)
                        for mt in range(NBLK):
                            dst = v_sb[:, mt, n * 512:(n + 1) * 512]
                            nc.scalar.copy(dst[:, 0:256], pss[mt][:, 0:256])
                            nc.vector.tensor_copy(dst[:, 256:512],
                                                  pss[mt][:, 256:512])

                # ---- K/Q projections with fused RoPE eviction ----
                def rope_evict(ps, dst_ap, h, m, bias_sb):
                    ts = slice(h * 512, (h + 1) * 512)
                    qa = rope_pool.tile([128, 512], BF16, tag="qa")
                    if bias_sb is not None:
                        nc.scalar.add(qa[:], ps[:], bias_sb[:, m:m + 1])
                    else:
                        nc.scalar.copy(qa[:], ps[:])
                    qsh = rope_pool.tile([128, 512], BF16, tag="qsh")
                    nc.sync.dma_start(qsh[0:64, :], qa[64:128, :])
                    nc.sync.dma_start(qsh[64:128, :], qa[0:64, :])
                    t1 = rope_pool.tile([128, 512], BF16, tag="t1")
                    nc.vector.tensor_mul(t1[:], qa[:], cos_sb[:, ts])
                    t2 = rope_pool.tile([128, 512], BF16, tag="t2")
                    nc.vector.tensor_mul(t2[:], qsh[:], sin_sb[:, ts])
                    nc.vector.tensor_add(dst_ap, t1[:], t2[:])

                # ---- K proj, then Q proj with woven attention ----
                # The attention pipeline rides inside the Q-projection
                # stream: one attention step is emitted after each Q psum
                # chain (64 chains <-> 64 steps, offset 9 so group g's
                # attention runs while group g+1 projects). Every attention
                # operand is then ~7us old when the PE reaches it, and the
                # ACT Exp<->Reciprocal table reloads (1.3us each) hide
                # entirely behind projection matmuls. Attention output is
                # normalized in-place into qT (each slice is dead as a query
                # once its s-matmul has run), which keeps SBUF under budget.
                with (
                    tc.tile_pool(name="psqk", bufs=2, space="PSUM") as psqk,
                    tc.tile_pool(name="ps_s", bufs=2, space="PSUM") as ps_s,
                    tc.tile_pool(name="ps_l", bufs=2, space="PSUM") as ps_l,
                    tc.tile_pool(name="ps_pv", bufs=1, space="PSUM") as ps_pv,
                    tc.tile_pool(name="ps_bc", bufs=1, space="PSUM") as ps_bc,
                    tc.tile_pool(name="attn", bufs=6) as attn_pool,
                    tc.tile_pool(name="bcbp", bufs=2) as bcb_pool,
                    tc.tile_pool(name="rcp", bufs=3) as rc_pool,
                ):
                    for m in range(MK):
                        wsb = wqk_pool.tile([128, KO, 128], BF16, tag="w")
                        nc.sync.dma_start(wsb[:], wk_t[m])
                        for h in range(2):
                            ps = psqk.tile([128, 512], F32, tag="ps")
                            for ko in range(KO):
                                nc.tensor.matmul(
                                    ps[:], wsb[:, ko, :],
                                    xt[:, ko, h * 512:(h + 1) * 512],
                                    start=(ko == 0), stop=(ko == KO - 1))
                            rope_evict(ps, kT[:, m, h * 512:(h + 1) * 512],
                                       h, m, bk_sb if use_bias else None)

                    iters = [(g, blk) for g in range(HKV)
                             for blk in range(NBLK)]
                    NIT = len(iters)
                    LAG = 5
                    st_wT = {}
                    st_rc = {}
                    l_cur = {}

                    def attn_step(i):
                        # stage A: scores + exp for iteration i
                        if i < NIT:
                            g, blk = iters[i]
                            tq = slice(blk * 128, (blk + 1) * 128)
                            s_ps = ps_s.tile([128, 512], F32, tag="s")
                            nc.tensor.matmul(
                                s_ps[:], kT[:, g, tq],
                                qT[:, g * G:(g + 1) * G, tq],
                                start=True, stop=True)
                            wT = attn_pool.tile([128, 512], BF16, tag="wT")
                            nc.scalar.activation(
                                out=wT[:], in_=s_ps[:],
                                func=mybir.ActivationFunctionType.Exp,
                                scale=SCALE, bias=mb_sb[:, blk:blk + 1])
                            st_wT[i] = wT
                        # stage B: row-sum; adjacent reciprocal pair per 4
                        j = i - 1
                        if 0 <= j < NIT:
                            b, r = divmod(j, 2)
                            if r == 0:
                                l_cur[b] = ps_l.tile([128, 512], F32,
                                                     tag="l", name=f"l{b}")
                            nc.tensor.matmul(
                                l_cur[b][32 * r:32 * r + 1, :], ones_col[:],
                                st_wT[j][:], start=True, stop=True)
                            if r == 1 and (b % 2 == 1 or j == NIT - 1):
                                for bb in ([b - 1, b] if b % 2 == 1 else [b]):
                                    rc = rc_pool.tile([64, 512], F32,
                                                      tag="rc", name=f"rc{bb}")
                                    _act_recip(nc, rc[:], l_cur[bb][0:64, :])
                                    st_rc[bb] = rc
                                    del l_cur[bb]
                        # stage C: pv + normalize broadcast + in-place store
                        j = i - LAG
                        if 0 <= j < NIT:
                            g, blk = iters[j]
                            b, r = divmod(j, 2)
                            pv_ps = ps_pv.tile([128, 512], F32, tag="pv")
                            nc.tensor.matmul(
                                pv_ps[:],
                                v_sb[:, blk, g * 128:(g + 1) * 128],
                                st_wT.pop(j)[:], start=True, stop=True)
                            bc_ps = ps_bc.tile([128, 512], F32, tag="bc")
                            nc.tensor.matmul(
                                bc_ps[:], ones_row_f[32 * r:32 * r + 1, :],
                                st_rc[b][32 * r:32 * r + 1, :],
                                start=True, stop=True)
                            if r == 1:
                                del st_rc[b]
                            bcb = bcb_pool.tile([128, 512], BF16, tag="bcb")
                            nc.vector.tensor_copy(bcb[:], bc_ps[:])
                            nc.vector.tensor_mul(
                                qT[:, g * G:(g + 1) * G,
                                   blk * 128:(blk + 1) * 128],
                                pv_ps[:], bcb[:])

                    chain = 0
                    for m in range(MQ):
                        wsb = wqk_pool.tile([128, KO, 128], BF16, tag="w")
                        nc.sync.dma_start(wsb[:], wq_t[m])
                        for h in range(2):
                            ps = psqk.tile([128, 512], F32, tag="ps")
                            for ko in range(KO):
                                nc.tensor.matmul(
                                    ps[:], wsb[:, ko, :],
                                    xt[:, ko, h * 512:(h + 1) * 512],
                                    start=(ko == 0), stop=(ko == KO - 1))
                            rope_evict(ps, qT[:, m, h * 512:(h + 1) * 512],
                                       h, m, bq_sb if use_bias else None)
                            chain += 1
                            if chain >= 10:
                                attn_step(chain - 10)
                    # tail: group 7's attention alone is ACT-bound (~50%
                    # PE duty), which lets HAM re-throttle and the out
                    # projection then starts at half clock. Weave the first
                    # out-projection chains (n=0, token blocks 0-1, using
                    # the now-idle psqk banks) into the tail steps as real
                    # PE work: the tail stays dense and out-proj effectively
                    # starts during it. Blocks 0-1 of n=0 are then skipped
                    # in the main out-proj loop.
                    pssA = [psqk.tile([128, 512], F32, tag="ps",
                                      name=f"psA{t}") for t in range(2)]
                    hk_done = 0
                    tail = list(range(chain - 9, NIT + LAG))
                    for idx, i in enumerate(tail):
                        attn_step(i)
                        target = min(MQ, (idx + 1) * 4)
                        if idx == len(tail) - 1:
                            target = MQ
                        while hk_done < target:
                            hp = hk_done // 2
                            wo_sb = wmov_pool.tile([128, 2, 512], BF16,
                                                   tag="wv")
                            nc.sync.dma_start(wo_sb[:], wo_t[hp, 0])
                            for k2 in range(2):
                                hk = 2 * hp + k2
                                for t in range(2):
                                    nc.tensor.matmul(
                                        pssA[t][:],
                                        qT[:, hk, t * 128:(t + 1) * 128],
                                        wo_sb[:, k2, :], start=(hk == 0),
                                        stop=(hk == MQ - 1 and not use_bias))
                            hk_done += 2
                    if use_bias:
                        for t in range(2):
                            nc.tensor.matmul(pssA[t][:], ones_row[:],
                                             bo_sb[:, 0, :],
                                             start=False, stop=True)
                    for t in range(2):
                        oeA = bcb_pool.tile([128, 512], BF16, tag="bcb",
                                            name=f"oeA{t}")
                        nc.scalar.copy(oeA[:, 0:256], pssA[t][:, 0:256])
                        nc.vector.tensor_copy(oeA[:, 256:512],
                                              pssA[t][:, 256:512])
                        nc.sync.dma_start(
                            out[t * 128:(t + 1) * 128, 0:512], oeA[:])

            # ================= out projection =================
            with (
                tc.tile_pool(name="wop", bufs=8) as wo_pool,
                tc.tile_pool(name="oe", bufs=6) as oe_pool,
                tc.tile_pool(name="psO", bufs=8, space="PSUM") as psO,
            ):
                for n in range(NE):
                    mts = list(range(2, NBLK)) if n == 0 else list(range(NBLK))
                    pss = {mt: psO.tile([128, 512], F32, tag="ps",
                                        name=f"pso{n}_{mt}")
                           for mt in mts}
                    for hp in range(MQ // 2):
                        wo_sb = wo_pool.tile([128, 2, 512], BF16, tag="wo")
                        nc.sync.dma_start(wo_sb[:], wo_t[hp, n])
                        for k2 in range(2):
                            hk = 2 * hp + k2
                            for mt in mts:
                                nc.tensor.matmul(
                                    pss[mt][:],
                                    qT[:, hk, mt * 128:(mt + 1) * 128],
                                    wo_sb[:, k2, :], start=(hk == 0),
                                    stop=(hk == MQ - 1 and not use_bias))
                    if use_bias:
                        for mt in mts:
                            nc.tensor.matmul(pss[mt][:], ones_row[:],
                                             bo_sb[:, n, :],
                                             start=False, stop=True)
                    for mt in mts:
                        oe = oe_pool.tile([128, 512], BF16, tag="oe")
                        nc.scalar.copy(oe[:, 0:256], pss[mt][:, 0:256])
                        nc.vector.tensor_copy(oe[:, 256:512],
                                              pss[mt][:, 256:512])
                        nc.sync.dma_start(
                            out[mt * 128:(mt + 1) * 128,
                                n * 512:(n + 1) * 512], oe[:])

    return _split_excess_waits(nc)


_NC_CACHE = {}


def _get_nc(use_bias: bool):
    if use_bias not in _NC_CACHE:
        _NC_CACHE[use_bias] = _build(use_bias)
    return _NC_CACHE[use_bias]


def _prepare(x, wq, bq, wk, bk, wv, bv, wo, bo, mask):
    x = np.asarray(x, np.float32)
    wq = np.asarray(wq, np.float32)
    wk = np.asarray(wk, np.float32)
    wv = np.asarray(wv, np.float32)
    wo = np.asarray(wo, np.float32)
    bq = np.asarray(bq, np.float32)
    bk = np.asarray(bk, np.float32)
    bv = np.asarray(bv, np.float32)
    bo = np.asarray(bo, np.float32)
    mask = np.asarray(mask)

    use_bias = bool(bq.any() or bk.any() or bv.any() or bo.any())

    # weight layouts (shared across cores)
    wq_t = np.ascontiguousarray(
        wq.reshape(KO, 128, MQ, 128).transpose(2, 1, 0, 3)).astype(NPBF16)
    wk_t = np.ascontiguousarray(
        wk.reshape(KO, 128, MK, 128).transpose(2, 1, 0, 3)).astype(NPBF16)
    wv_t = np.ascontiguousarray(
        wv.reshape(KO // 2, 2, 128, 2, 512).transpose(0, 3, 2, 1, 4)).astype(NPBF16)
    wo_t = np.ascontiguousarray(
        wo.reshape(MQ // 2, 2, 128, NE, 512).transpose(0, 3, 2, 1, 4)).astype(NPBF16)

    # RoPE tables (positions are global sequence positions)
    inv = 1.0 / (ROPE_BASE ** (np.arange(0, D, 2, dtype=np.float32) / D))
    pos = np.arange(S, dtype=np.float32)
    ang = pos[:, None] * inv[None, :]                      # [S, 64]
    cos_full = np.concatenate([np.cos(ang), np.cos(ang)], -1).T  # [128, S]
    sin_half = np.sin(ang).T                               # [64, S]
    sin_eff = np.concatenate([-sin_half, sin_half], 0)     # [128, S]

    shards_per_b = NCORES // B                             # 4
    in_maps = []
    for c in range(NCORES):
        b = c // shards_per_b
        s0 = (c % shards_per_b) * TOK
        xs = x[b, s0:s0 + TOK]                             # [TOK, E]
        xT_t = np.ascontiguousarray(xs.T).astype(NPBF16).reshape(KO, 128, TOK)
        mshard = mask[b, s0:s0 + TOK].reshape(NBLK, BS)
        mb = np.where(mshard, np.float32(0.0), np.float32(-80.0)).astype(np.float32)
        im = {
            "xT": xT_t,
            "wq_t": wq_t, "wk_t": wk_t, "wv_t": wv_t, "wo_t": wo_t,
            "cos_t": np.ascontiguousarray(cos_full[:, s0:s0 + TOK]).astype(NPBF16),
            "sin_t": np.ascontiguousarray(sin_eff[:, s0:s0 + TOK]).astype(NPBF16),
            "mb_t": mb,
        }
        if use_bias:
            im["bq_t"] = bq.reshape(MQ, 128).copy()
            im["bk_t"] = bk.reshape(MK, 128).copy()
            im["bv_t"] = bv.reshape(2, 512).astype(NPBF16)
            im["bo_t"] = bo.reshape(NE, 512).astype(NPBF16)
        in_maps.append(im)

    return in_maps, use_bias


def _assemble(results):
    shards_per_b = NCORES // B
    out = np.empty((B, S, E), np.float32)
    for c in range(NCORES):
        b = c // shards_per_b
        s0 = (c % shards_per_b) * TOK
        out[b, s0:s0 + TOK] = np.asarray(results[c]["out"], np.float32)
    return out


def kernel(**inputs):
    in_maps, use_bias = _prepare(**inputs)
    nc = _get_nc(use_bias)
    res = run_bass_kernel_spmd(nc, in_maps, core_ids=list(range(NCORES)))
    return _assemble(res.results)


# revision 22
# speedup vs baseline: 1.0354x; 1.0030x over previous
"""Trainium2 Bass kernel for CosmicMultiHeadAttention (block-local flash attention).

Sharding: the 8192 tokens (B=2 x S=4096) are split into 8 shards of 1024
tokens (batch-major). Attention is block-local with 128-token blocks, so
1024-token shards (8 blocks each) have zero cross-shard dependencies: every
core runs the full layer (QKV proj + RoPE + block attention + out proj) for
its own tokens. No collectives.

v2 layout strategy (single pass over all 1024 tokens, weights loaded once):
  - x is transposed on the host to xT [E, tok] so the E contraction sits on
    the partition axis; resident in SBUF for the whole projection phase.
  - V proj first: its ko-loop matmuls are paced by the arriving xt DMAs, so
    the PE starts ~3us in and stays busy through the initial load.
    Swapped operands (lhsT = xt tiles) land v natural [tok, hd] - exactly
    the PV-matmul lhsT layout.
  - K/Q projections: lhsT = weight tiles (1MB DMA each, loaded ONCE),
    rhs = xt -> psum kT/qT [hd, tok]. RoPE applied during psum eviction
    (ACT copy + partition-shift DMA + DVE).
  - attention: 64 iterations (8 blocks x 8 kv-heads, 4 grouped q-heads,
    N=512), 3-stage software pipeline across engines:
      stage1(i):   s = kT.T @ qT;  ACT exp (scale, mask-bias) -> wT (SBUF)
      stage2(i-1): l = ones.T @ wT; DVE recip; pv = v.T @ wT
      stage3(i-2): bc = broadcast(1/l) via K=1 matmul; DVE mul -> oT
    PE order per step: s(i), l(i-1), pv(i-1), bc(i-2) - every operand was
    produced >= 1 full step (~850ns) earlier, so the PE never waits on the
    ACT-exp / DVE-recip chains and HAM stays at K=8/8.
  - out projection: lhsT = oT tiles (on-chip), rhs = wo tiles (loaded once),
    8 psum banks accumulate over the 32 head k-tiles, evictions alternate
    ACT/DVE so the next n-tile's first matmuls aren't serialized behind a
    single engine.
"""

import sys

if '/opt/trn_rl_repo' not in sys.path:
    sys.path.insert(0, '/opt/trn_rl_repo')

import numpy as np
import ml_dtypes

import concourse.bass as bass
import concourse.tile as tile
from concourse import mybir
from concourse.bass_utils import run_bass_kernel_spmd

BF16 = mybir.dt.bfloat16
F32 = mybir.dt.float32
NPBF16 = ml_dtypes.bfloat16

B, S, E = 2, 4096, 4096
HQ, HKV, D = 32, 8, 128
BS = 128
ROPE_BASE = 10000.0
NCORES = 8
TOK = (B * S) // NCORES          # 1024 tokens per core
KO = E // 128                    # 32 k-tiles over E
MQ = (HQ * D) // 128             # 32 q head-tiles
MK = (HKV * D) // 128            # 8 k head-tiles
G = HQ // HKV                    # 4 q heads per kv head
NBLK = TOK // BS                 # 8 blocks per core
NE = E // 512                    # 8 out-proj column tiles
SCALE = 1.0 / float(np.sqrt(D))


# ---------------------------------------------------------------------------
# The walrus build in this image rejects instructions carrying more than one
# "sem-ge" sync wait ("Too many sync wait commands"; Drain/CTRL accepts
# none). Tile's scheduler freely attaches several. Post-pass: keep at most
# one ge-wait per instruction (none on Drain) and move the excess onto
# EventSemaphore carrier instructions inserted immediately before, on the
# same engine - program order preserves the blocking semantics exactly.
# ---------------------------------------------------------------------------
def _split_excess_waits(nc):
    import bass_rust
    ctr = 0
    for f in nc.m.functions:
        for bb in f.blocks:
            out_list = []
            for inst in bb.instructions:
                si = inst.sync_info
                all_waits = list(si.on_wait) if si and si.on_wait else []
                ge = [w for w in all_waits if 'ge' in w.wait_mode]
                eq = [w for w in all_waits if 'ge' not in w.wait_mode]
                keep_n = 0 if type(inst).__name__ == 'InstDrain' else 1
                if len(ge) > keep_n:
                    extra, keep = ge[:-keep_n] if keep_n else ge, \
                        ge[-keep_n:] if keep_n else []
                    for w in extra:
                        ctr += 1
                        es = mybir.InstEventSemaphore(
                            name=f'waitsplit_{ctr}', engine=inst.engine,
                            ins=[], outs=[],
                            sync_info=bass_rust.SyncInfo(
                                on_wait=[w], on_update=[]))
                        out_list.append(es)
                    si.on_wait = eq + keep
                out_list.append(inst)
            bb.instructions[:] = out_list
    return nc


def _act_recip(nc, out_ap, in_ap):
    # ACT LUT reciprocal, emitted directly: the bass wrapper refuses
    # Reciprocal on accuracy grounds, but ~1e-3 relative is ample here and
    # every DVE reciprocal path is either ~6 passes (3.3us per call) or
    # unsupported by this walrus build.
    se = nc.scalar
    ins = [se.lower_ap(in_ap)]
    for arg in (0.0, 1.0, 0.0):  # bias, scale, alpha
        ins.append(mybir.ImmediateValue(dtype=mybir.dt.float32, value=arg))
    return se.add_instruction(
        mybir.InstActivation(
            name=se.bass.get_next_instruction_name(),
            func=mybir.ActivationFunctionType.Reciprocal,
            ins=ins, outs=[se.lower_ap(out_ap)]))


def _build(use_bias: bool):
    nc = bass.Bass()

    xT = nc.dram_tensor("xT", [KO, 128, TOK], BF16, kind="ExternalInput")
    wq_t = nc.dram_tensor("wq_t", [MQ, 128, KO, 128], BF16, kind="ExternalInput")
    wk_t = nc.dram_tensor("wk_t", [MK, 128, KO, 128], BF16, kind="ExternalInput")
    wv_t = nc.dram_tensor("wv_t", [KO // 2, 2, 128, 2, 512], BF16, kind="ExternalInput")
    wo_t = nc.dram_tensor("wo_t", [MQ // 2, NE, 128, 2, 512], BF16, kind="ExternalInput")
    cos_t = nc.dram_tensor("cos_t", [128, TOK], BF16, kind="ExternalInput")
    sin_t = nc.dram_tensor("sin_t", [128, TOK], BF16, kind="ExternalInput")
    mb_t = nc.dram_tensor("mb_t", [NBLK, 128], F32, kind="ExternalInput")
    if use_bias:
        bq_t = nc.dram_tensor("bq_t", [MQ, 128], F32, kind="ExternalInput")
        bk_t = nc.dram_tensor("bk_t", [MK, 128], F32, kind="ExternalInput")
        bv_t = nc.dram_tensor("bv_t", [2, 512], BF16, kind="ExternalInput")
        bo_t = nc.dram_tensor("bo_t", [NE, 512], BF16, kind="ExternalInput")
    out = nc.dram_tensor("out", [TOK, E], BF16, kind="ExternalOutput")

    with tile.TileContext(nc) as tc:
        with (
            tc.tile_pool(name="const", bufs=1) as cpool,
            tc.tile_pool(name="keep", bufs=1) as keep,
        ):
            # ---- constants (DMAs issued after xt below; xt is critical path) ----
            cos_sb = cpool.tile([128, TOK], BF16, tag="cos")
            sin_sb = cpool.tile([128, TOK], BF16, tag="sin")
            mb_sb = cpool.tile([128, NBLK], F32, tag="mb")
            ones_col = cpool.tile([128, 1], BF16, tag="ones_col")
            nc.vector.memset(ones_col[:], 1.0)
            ones_row_f = cpool.tile([64, 128], F32, tag="ones_row_f")
            nc.vector.memset(ones_row_f[:], 1.0)
            if use_bias:
                ones_row = cpool.tile([1, 128], BF16, tag="ones_row")
                nc.vector.memset(ones_row[:], 1.0)
                bq_sb = cpool.tile([128, MQ], F32, tag="bq")
                nc.sync.dma_start(bq_sb[:], bq_t.rearrange("m p -> p m"))
                bk_sb = cpool.tile([128, MK], F32, tag="bk")
                nc.sync.dma_start(bk_sb[:], bk_t.rearrange("m p -> p m"))
                bv_sb = cpool.tile([1, 2, 512], BF16, tag="bv")
                nc.sync.dma_start(bv_sb[:], bv_t[None, :, :])
                bo_sb = cpool.tile([1, NE, 512], BF16, tag="bo")
                nc.sync.dma_start(bo_sb[:], bo_t[None, :, :])

            # persistent intermediates
            qT = keep.tile([128, MQ, TOK], BF16, tag="qT")
            kT = keep.tile([128, MK, TOK], BF16, tag="kT")
            v_sb = keep.tile([128, NBLK, HKV * D], BF16, tag="v")

            # ================= projection phase =================
            with (
                tc.tile_pool(name="xtp", bufs=1) as xt_pool,
                tc.tile_pool(name="wmov", bufs=2) as wmov_pool,
            ):
                # The n=0 V weights live in one upfront tile whose DMAs are
                # interleaved with the xt slices in consumption order: DMA
                # queues are FIFO, so anything the V loop needs early must
                # also be ISSUED early, and ring-buffered tiles cannot be
                # issued far ahead (their WAR semaphore would stall the
                # queue). Supply measures ~380 GB/s once ramped, so this
                # keeps the V projection compute-bound from ~10us.
                xt = xt_pool.tile([128, KO, TOK], BF16, tag="xt")
                with tc.tile_pool(name="wv0", bufs=1) as wv0_pool:
                    wvall = wv0_pool.tile([128, KO // 2, 2, 512], BF16,
                                          tag="wvall")
                    for kp in range(KO // 2):
                        nc.sync.dma_start(wvall[:, kp, :, :], wv_t[kp, 0])
                        nc.sync.dma_start(xt[:, 2 * kp, :], xT[2 * kp])
                        nc.sync.dma_start(xt[:, 2 * kp + 1, :], xT[2 * kp + 1])
                    nc.sync.dma_start(cos_sb[:], cos_t[:, :])
                    nc.sync.dma_start(sin_sb[:], sin_t[:, :])
                    nc.sync.dma_start(mb_sb[:], mb_t.rearrange("b p -> p b"))

                    # ---- V projection (paced by xt arrival) ----
                    with tc.tile_pool(name="psv", bufs=8, space="PSUM") as psv:
                        for n in range(2):
                            pss = [psv.tile([128, 512], F32, tag="ps",
                                            name=f"psv{n}_{mt}")
                                   for mt in range(NBLK)]
                            for kp in range(KO // 2):
                                if n == 0:
                                    wv_sb = wvall[:, kp, :, :]
                                else:
                                    wv_t2 = wmov_pool.tile(
                                        [128, 2, 512], BF16, tag="wv")
                                    nc.sync.dma_start(wv_t2[:], wv_t[kp, n])
                                    wv_sb = wv_t2[:]
                                for k2 in range(2):
                                    ko = 2 * kp + k2
                                    for mt in range(NBLK):
                                        nc.tensor.matmul(
                                            pss[mt][:],
                                            xt[:, ko, mt * 128:(mt + 1) * 128],
                                            wv_sb[:, k2, :], start=(ko == 0),
                                            stop=(ko == KO - 1 and not use_bias))
                            if use_bias:
                                for mt in range(NBLK):
                                    nc.tensor.matmul(pss[mt][:], ones_row[:],
                                                     bv_sb[:, n, :],
                                                     start=False, stop=True)
                            for mt in range(NBLK):
                                dst = v_sb[:, mt, n * 512:(n + 1) * 512]
                                nc.scalar.copy(dst[:, 0:256], pss[mt][:, 0:256])
                                nc.vector.tensor_copy(dst[:, 256:512],
                                                      pss[mt][:, 256:512])

                # ---- K/Q projections with fused RoPE eviction ----
                def rope_evict(ps, dst_ap, h, m, bias_sb):
                    ts = slice(h * 512, (h + 1) * 512)
                    qa = rope_pool.tile([128, 512], BF16, tag="qa")
                    if bias_sb is not None:
                        nc.scalar.add(qa[:], ps[:], bias_sb[:, m:m + 1])
                    else:
                        nc.scalar.copy(qa[:], ps[:])
                    qsh = rope_pool.tile([128, 512], BF16, tag="qsh")
                    nc.sync.dma_start(qsh[0:64, :], qa[64:128, :])
                    nc.sync.dma_start(qsh[64:128, :], qa[0:64, :])
                    t1 = rope_pool.tile([128, 512], BF16, tag="t1")
                    nc.vector.tensor_mul(t1[:], qa[:], cos_sb[:, ts])
                    t2 = rope_pool.tile([128, 512], BF16, tag="t2")
                    nc.vector.tensor_mul(t2[:], qsh[:], sin_sb[:, ts])
                    nc.vector.tensor_add(dst_ap, t1[:], t2[:])

                # ---- K proj, then Q proj with woven attention ----
                # The attention pipeline rides inside the Q-projection
                # stream: one attention step is emitted after each Q psum
                # chain (64 chains <-> 64 steps, offset 9 so group g's
                # attention runs while group g+1 projects). Every attention
                # operand is then ~7us old when the PE reaches it, and the
                # ACT Exp<->Reciprocal table reloads (1.3us each) hide
                # entirely behind projection matmuls. Attention output is
                # normalized in-place into qT (each slice is dead as a query
                # once its s-matmul has run), which keeps SBUF under budget.
                with (
                    tc.tile_pool(name="wqk", bufs=2) as wqk_pool,
                    tc.tile_pool(name="rope", bufs=2) as rope_pool,
                    tc.tile_pool(name="psqk", bufs=2, space="PSUM") as psqk,
                    tc.tile_pool(name="ps_s", bufs=2, space="PSUM") as ps_s,
                    tc.tile_pool(name="ps_l", bufs=2, space="PSUM") as ps_l,
                    tc.tile_pool(name="ps_pv", bufs=1, space="PSUM") as ps_pv,
                    tc.tile_pool(name="ps_bc", bufs=1, space="PSUM") as ps_bc,
                    tc.tile_pool(name="attn", bufs=6) as attn_pool,
                    tc.tile_pool(name="bcbp", bufs=2) as bcb_pool,
                    tc.tile_pool(name="rcp", bufs=3) as rc_pool,
                ):
                    for m in range(MK):
                        wsb = wqk_pool.tile([128, KO, 128], BF16, tag="w")
                        nc.sync.dma_start(wsb[:], wk_t[m])
                        for h in range(2):
                            ps = psqk.tile([128, 512], F32, tag="ps")
                            for ko in range(KO):
                                nc.tensor.matmul(
                                    ps[:], wsb[:, ko, :],
                                    xt[:, ko, h * 512:(h + 1) * 512],
                                    start=(ko == 0), stop=(ko == KO - 1))
                            rope_evict(ps, kT[:, m, h * 512:(h + 1) * 512],
                                       h, m, bk_sb if use_bias else None)

                    iters = [(g, blk) for g in range(HKV)
                             for blk in range(NBLK)]
                    NIT = len(iters)
                    LAG = 5
                    st_wT = {}
                    st_rc = {}
                    l_cur = {}

                    def attn_step(i):
                        # stage A: scores + exp for iteration i
                        if i < NIT:
                            g, blk = iters[i]
                            tq = slice(blk * 128, (blk + 1) * 128)
                            s_ps = ps_s.tile([128, 512], F32, tag="s")
                            nc.tensor.matmul(
                                s_ps[:], kT[:, g, tq],
                                qT[:, g * G:(g + 1) * G, tq],
                                start=True, stop=True)
                            wT = attn_pool.tile([128, 512], BF16, tag="wT")
                            nc.scalar.activation(
                                out=wT[:], in_=s_ps[:],
                                func=mybir.ActivationFunctionType.Exp,
                                scale=SCALE, bias=mb_sb[:, blk:blk + 1])
                            st_wT[i] = wT
                        # stage B: row-sum; adjacent reciprocal pair per 4
                        j = i - 1
                        if 0 <= j < NIT:
                            b, r = divmod(j, 2)
                            if r == 0:
                                l_cur[b] = ps_l.tile([128, 512], F32,
                                                     tag="l", name=f"l{b}")
                            nc.tensor.matmul(
                                l_cur[b][32 * r:32 * r + 1, :], ones_col[:],
                                st_wT[j][:], start=True, stop=True)
                            if r == 1 and (b % 2 == 1 or j == NIT - 1):
                                for bb in ([b - 1, b] if b % 2 == 1 else [b]):
                                    rc = rc_pool.tile([64, 512], F32,
                                                      tag="rc", name=f"rc{bb}")
                                    _act_recip(nc, rc[:], l_cur[bb][0:64, :])
                                    st_rc[bb] = rc
                                    del l_cur[bb]
                        # stage C: pv + normalize broadcast + in-place store
                        j = i - LAG
                        if 0 <= j < NIT:
                            g, blk = iters[j]
                            b, r = divmod(j, 2)
                            pv_ps = ps_pv.tile([128, 512], F32, tag="pv")
                            nc.tensor.matmul(
                                pv_ps[:],
                                v_sb[:, blk, g * 128:(g + 1) * 128],
                                st_wT.pop(j)[:], start=True, stop=True)
                            bc_ps = ps_bc.tile([128, 512], F32, tag="bc")
                            nc.tensor.matmul(
                                bc_ps[:], ones_row_f[32 * r:32 * r + 1, :],
                                st_rc[b][32 * r:32 * r + 1, :],
                                start=True, stop=True)
                            if r == 1:
                                del st_rc[b]
                            bcb = bcb_pool.tile([128, 512], BF16, tag="bcb")
                            nc.vector.tensor_copy(bcb[:], bc_ps[:])
                            nc.vector.tensor_mul(
                                qT[:, g * G:(g + 1) * G,
                                   blk * 128:(blk + 1) * 128],
                                pv_ps[:], bcb[:])

                    chain = 0
                    for m in range(MQ):
                        wsb = wqk_pool.tile([128, KO, 128], BF16, tag="w")
                        nc.sync.dma_start(wsb[:], wq_t[m])
                        for h in range(2):
                            ps = psqk.tile([128, 512], F32, tag="ps")
                            for ko in range(KO):
                                nc.tensor.matmul(
                                    ps[:], wsb[:, ko, :],
                                    xt[:, ko, h * 512:(h + 1) * 512],
                                    start=(ko == 0), stop=(ko == KO - 1))
                            rope_evict(ps, qT[:, m, h * 512:(h + 1) * 512],
                                       h, m, bq_sb if use_bias else None)
                            chain += 1
                            if chain >= 10:
                                attn_step(chain - 10)
                    # tail: group 7's attention alone is ACT-bound (~50%
                    # PE duty), which lets HAM re-throttle and the out
                    # projection then starts at half clock. Weave the first
                    # out-projection chains (n=0, token blocks 0-1, using
                    # the now-idle psqk banks) into the tail steps as real
                    # PE work: the tail stays dense and out-proj effectively
                    # starts during it. Blocks 0-1 of n=0 are then skipped
                    # in the main out-proj loop.
                    pssA = [psqk.tile([128, 512], F32, tag="ps",
                                      name=f"psA{t}") for t in range(2)]
                    hk_done = 0
                    tail = list(range(chain - 9, NIT + LAG))
                    for idx, i in enumerate(tail):
                        attn_step(i)
                        target = min(MQ, (idx + 1) * 4)
                        if idx == len(tail) - 1:
                            target = MQ
                        while hk_done < target:
                            hp = hk_done // 2
                            wo_sb = wmov_pool.tile([128, 2, 512], BF16,
                                                   tag="wv")
                            nc.sync.dma_start(wo_sb[:], wo_t[hp, 0])
                            for k2 in range(2):
                                hk = 2 * hp + k2
                                for t in range(2):
                                    nc.tensor.matmul(
                                        pssA[t][:],
                                        qT[:, hk, t * 128:(t + 1) * 128],
                                        wo_sb[:, k2, :], start=(hk == 0),
                                        stop=(hk == MQ - 1 and not use_bias))
                            hk_done += 2
                    if use_bias:
                        for t in range(2):
                            nc.tensor.matmul(pssA[t][:], ones_row[:],
                                             bo_sb[:, 0, :],
                                             start=False, stop=True)
                    for t in range(2):
                        oeA = bcb_pool.tile([128, 512], BF16, tag="bcb",
                                            name=f"oeA{t}")
                        nc.scalar.copy(oeA[:, 0:256], pssA[t][:, 0:256])
                        nc.vector.tensor_copy(oeA[:, 256:512],
                                              pssA[t][:, 256:512])
                        nc.sync.dma_start(
                            out[t * 128:(t + 1) * 128, 0:512], oeA[:])

            # ================= out projection =================
            with (
                tc.tile_pool(name="wop", bufs=8) as wo_pool,
                tc.tile_pool(name="oe", bufs=6) as oe_pool,
                tc.tile_pool(name="psO", bufs=8, space="PSUM") as psO,
            ):
                for n in range(NE):
                    mts = list(range(2, NBLK)) if n == 0 else list(range(NBLK))
                    pss = {mt: psO.tile([128, 512], F32, tag="ps",
                                        name=f"pso{n}_{mt}")
                           for mt in mts}
                    for hp in range(MQ // 2):
                        wo_sb = wo_pool.tile([128, 2, 512], BF16, tag="wo")
                        nc.sync.dma_start(wo_sb[:], wo_t[hp, n])
                        for k2 in range(2):
                            hk = 2 * hp + k2
                            for mt in mts:
                                nc.tensor.matmul(
                                    pss[mt][:],
                                    qT[:, hk, mt * 128:(mt + 1) * 128],
                                    wo_sb[:, k2, :], start=(hk == 0),
                                    stop=(hk == MQ - 1 and not use_bias))
                    if use_bias:
                        for mt in mts:
                            nc.tensor.matmul(pss[mt][:], ones_row[:],
                                             bo_sb[:, n, :],
                                             start=False, stop=True)
                    for mt in mts:
                        oe = oe_pool.tile([128, 512], BF16, tag="oe")
                        nc.scalar.copy(oe[:, 0:256], pss[mt][:, 0:256])
                        nc.vector.tensor_copy(oe[:, 256:512],
                                              pss[mt][:, 256:512])
                        nc.sync.dma_start(
                            out[mt * 128:(mt + 1) * 128,
                                n * 512:(n + 1) * 512], oe[:])

    return _split_excess_waits(nc)


_NC_CACHE = {}


def _get_nc(use_bias: bool):
    if use_bias not in _NC_CACHE:
        _NC_CACHE[use_bias] = _build(use_bias)
    return _NC_CACHE[use_bias]


def _prepare(x, wq, bq, wk, bk, wv, bv, wo, bo, mask):
    x = np.asarray(x, np.float32)
    wq = np.asarray(wq, np.float32)
    wk = np.asarray(wk, np.float32)
    wv = np.asarray(wv, np.float32)
    wo = np.asarray(wo, np.float32)
    bq = np.asarray(bq, np.float32)
    bk = np.asarray(bk, np.float32)
    bv = np.asarray(bv, np.float32)
    bo = np.asarray(bo, np.float32)
    mask = np.asarray(mask)

    use_bias = bool(bq.any() or bk.any() or bv.any() or bo.any())

    # weight layouts (shared across cores)
    wq_t = np.ascontiguousarray(
        wq.reshape(KO, 128, MQ, 128).transpose(2, 1, 0, 3)).astype(NPBF16)
    wk_t = np.ascontiguousarray(
        wk.reshape(KO, 128, MK, 128).transpose(2, 1, 0, 3)).astype(NPBF16)
    wv_t = np.ascontiguousarray(
        wv.reshape(KO // 2, 2, 128, 2, 512).transpose(0, 3, 2, 1, 4)).astype(NPBF16)
    wo_t = np.ascontiguousarray(
        wo.reshape(MQ // 2, 2, 128, NE, 512).transpose(0, 3, 2, 1, 4)).astype(NPBF16)

    # RoPE tables (positions are global sequence positions)
    inv = 1.0 / (ROPE_BASE ** (np.arange(0, D, 2, dtype=np.float32) / D))
    pos = np.arange(S, dtype=np.float32)
    ang = pos[:, None] * inv[None, :]                      # [S, 64]
    cos_full = np.concatenate([np.cos(ang), np.cos(ang)], -1).T  # [128, S]
    sin_half = np.sin(ang).T                               # [64, S]
    sin_eff = np.concatenate([-sin_half, sin_half], 0)     # [128, S]

    shards_per_b = NCORES // B                             # 4
    in_maps = []
    for c in range(NCORES):
        b = c // shards_per_b
        s0 = (c % shards_per_b) * TOK
        xs = x[b, s0:s0 + TOK]                             # [TOK, E]
        xT_t = np.ascontiguousarray(xs.T).astype(NPBF16).reshape(KO, 128, TOK)
        mshard = mask[b, s0:s0 + TOK].reshape(NBLK, BS)
        mb = np.where(mshard, np.float32(0.0), np.float32(-80.0)).astype(np.float32)
        im = {
            "xT": xT_t,
            "wq_t": wq_t, "wk_t": wk_t, "wv_t": wv_t, "wo_t": wo_t,
            "cos_t": np.ascontiguousarray(cos_full[:, s0:s0 + TOK]).astype(NPBF16),
            "sin_t": np.ascontiguousarray(sin_eff[:, s0:s0 + TOK]).astype(NPBF16),
            "mb_t": mb,
        }
        if use_bias:
            im["bq_t"] = bq.reshape(MQ, 128).copy()
            im["bk_t"] = bk.reshape(MK, 128).copy()
            im["bv_t"] = bv.reshape(2, 512).astype(NPBF16)
            im["bo_t"] = bo.reshape(NE, 512).astype(NPBF16)
        in_maps.append(im)

    return in_maps, use_bias


def _assemble(results):
    shards_per_b = NCORES // B
    out = np.empty((B, S, E), np.float32)
    for c in range(NCORES):
        b = c // shards_per_b
        s0 = (c % shards_per_b) * TOK
        out[b, s0:s0 + TOK] = np.asarray(results[c]["out"], np.float32)
    return out


def kernel(**inputs):
    in_maps, use_bias = _prepare(**inputs)
    nc = _get_nc(use_bias)
    res = run_bass_kernel_spmd(nc, in_maps, core_ids=list(range(NCORES)))
    return _assemble(res.results)


# revision 25
# speedup vs baseline: 1.0474x; 1.0116x over previous
"""Trainium2 Bass kernel for CosmicMultiHeadAttention (block-local flash attention).

Sharding: the 8192 tokens (B=2 x S=4096) are split into 8 shards of 1024
tokens (batch-major). Attention is block-local with 128-token blocks, so
1024-token shards (8 blocks each) have zero cross-shard dependencies: every
core runs the full layer (QKV proj + RoPE + block attention + out proj) for
its own tokens. No collectives.

v2 layout strategy (single pass over all 1024 tokens, weights loaded once):
  - x is transposed on the host to xT [E, tok] so the E contraction sits on
    the partition axis; resident in SBUF for the whole projection phase.
  - V proj first: its ko-loop matmuls are paced by the arriving xt DMAs, so
    the PE starts ~3us in and stays busy through the initial load.
    Swapped operands (lhsT = xt tiles) land v natural [tok, hd] - exactly
    the PV-matmul lhsT layout.
  - K/Q projections: lhsT = weight tiles (1MB DMA each, loaded ONCE),
    rhs = xt -> psum kT/qT [hd, tok]. RoPE applied during psum eviction
    (ACT copy + partition-shift DMA + DVE).
  - attention: 64 iterations (8 blocks x 8 kv-heads, 4 grouped q-heads,
    N=512), 3-stage software pipeline across engines:
      stage1(i):   s = kT.T @ qT;  ACT exp (scale, mask-bias) -> wT (SBUF)
      stage2(i-1): l = ones.T @ wT; DVE recip; pv = v.T @ wT
      stage3(i-2): bc = broadcast(1/l) via K=1 matmul; DVE mul -> oT
    PE order per step: s(i), l(i-1), pv(i-1), bc(i-2) - every operand was
    produced >= 1 full step (~850ns) earlier, so the PE never waits on the
    ACT-exp / DVE-recip chains and HAM stays at K=8/8.
  - out projection: lhsT = oT tiles (on-chip), rhs = wo tiles (loaded once),
    8 psum banks accumulate over the 32 head k-tiles, evictions alternate
    ACT/DVE so the next n-tile's first matmuls aren't serialized behind a
    single engine.
"""

import sys

if '/opt/trn_rl_repo' not in sys.path:
    sys.path.insert(0, '/opt/trn_rl_repo')

import numpy as np
import ml_dtypes

import concourse.bass as bass
import concourse.tile as tile
from concourse import mybir
from concourse.bass_utils import run_bass_kernel_spmd

BF16 = mybir.dt.bfloat16
F32 = mybir.dt.float32
NPBF16 = ml_dtypes.bfloat16

B, S, E = 2, 4096, 4096
HQ, HKV, D = 32, 8, 128
BS = 128
ROPE_BASE = 10000.0
NCORES = 8
TOK = (B * S) // NCORES          # 1024 tokens per core
KO = E // 128                    # 32 k-tiles over E
MQ = (HQ * D) // 128             # 32 q head-tiles
MK = (HKV * D) // 128            # 8 k head-tiles
G = HQ // HKV                    # 4 q heads per kv head
NBLK = TOK // BS                 # 8 blocks per core
NE = E // 512                    # 8 out-proj column tiles
SCALE = 1.0 / float(np.sqrt(D))


# ---------------------------------------------------------------------------
# The walrus build in this image rejects instructions carrying more than one
# "sem-ge" sync wait ("Too many sync wait commands"; Drain/CTRL accepts
# none). Tile's scheduler freely attaches several. Post-pass: keep at most
# one ge-wait per instruction (none on Drain) and move the excess onto
# EventSemaphore carrier instructions inserted immediately before, on the
# same engine - program order preserves the blocking semantics exactly.
# ---------------------------------------------------------------------------
def _split_excess_waits(nc):
    import bass_rust
    ctr = 0
    for f in nc.m.functions:
        for bb in f.blocks:
            out_list = []
            for inst in bb.instructions:
                si = inst.sync_info
                all_waits = list(si.on_wait) if si and si.on_wait else []
                ge = [w for w in all_waits if 'ge' in w.wait_mode]
                eq = [w for w in all_waits if 'ge' not in w.wait_mode]
                keep_n = 0 if type(inst).__name__ == 'InstDrain' else 1
                if len(ge) > keep_n:
                    extra, keep = ge[:-keep_n] if keep_n else ge, \
                        ge[-keep_n:] if keep_n else []
                    for w in extra:
                        ctr += 1
                        es = mybir.InstEventSemaphore(
                            name=f'waitsplit_{ctr}', engine=inst.engine,
                            ins=[], outs=[],
                            sync_info=bass_rust.SyncInfo(
                                on_wait=[w], on_update=[]))
                        out_list.append(es)
                    si.on_wait = eq + keep
                out_list.append(inst)
            bb.instructions[:] = out_list
    return nc


def _act_recip(nc, out_ap, in_ap):
    # ACT LUT reciprocal, emitted directly: the bass wrapper refuses
    # Reciprocal on accuracy grounds, but ~1e-3 relative is ample here and
    # every DVE reciprocal path is either ~6 passes (3.3us per call) or
    # unsupported by this walrus build.
    se = nc.scalar
    ins = [se.lower_ap(in_ap)]
    for arg in (0.0, 1.0, 0.0):  # bias, scale, alpha
        ins.append(mybir.ImmediateValue(dtype=mybir.dt.float32, value=arg))
    return se.add_instruction(
        mybir.InstActivation(
            name=se.bass.get_next_instruction_name(),
            func=mybir.ActivationFunctionType.Reciprocal,
            ins=ins, outs=[se.lower_ap(out_ap)]))


def _build(use_bias: bool):
    nc = bass.Bass()

    xT = nc.dram_tensor("xT", [KO, 128, TOK], BF16, kind="ExternalInput")
    wq_t = nc.dram_tensor("wq_t", [MQ, 128, KO, 128], BF16, kind="ExternalInput")
    wk_t = nc.dram_tensor("wk_t", [MK, 128, KO, 128], BF16, kind="ExternalInput")
    wv_t = nc.dram_tensor("wv_t", [KO // 2, 2, 128, 2, 512], BF16, kind="ExternalInput")
    wo_t = nc.dram_tensor("wo_t", [MQ // 2, NE, 128, 2, 512], BF16, kind="ExternalInput")
    cos_t = nc.dram_tensor("cos_t", [128, TOK], BF16, kind="ExternalInput")
    sin_t = nc.dram_tensor("sin_t", [128, TOK], BF16, kind="ExternalInput")
    mb_t = nc.dram_tensor("mb_t", [NBLK, 128], F32, kind="ExternalInput")
    if use_bias:
        bq_t = nc.dram_tensor("bq_t", [MQ, 128], F32, kind="ExternalInput")
        bk_t = nc.dram_tensor("bk_t", [MK, 128], F32, kind="ExternalInput")
        bv_t = nc.dram_tensor("bv_t", [2, 512], BF16, kind="ExternalInput")
        bo_t = nc.dram_tensor("bo_t", [NE, 512], BF16, kind="ExternalInput")
    out = nc.dram_tensor("out", [TOK, E], BF16, kind="ExternalOutput")

    with tile.TileContext(nc) as tc:
        with (
            tc.tile_pool(name="const", bufs=1) as cpool,
            tc.tile_pool(name="keep", bufs=1) as keep,
        ):
            # ---- constants (DMAs issued after xt below; xt is critical path) ----
            cos_sb = cpool.tile([128, TOK], BF16, tag="cos")
            sin_sb = cpool.tile([128, TOK], BF16, tag="sin")
            mb_sb = cpool.tile([128, NBLK], F32, tag="mb")
            ones_col = cpool.tile([128, 1], BF16, tag="ones_col")
            nc.vector.memset(ones_col[:], 1.0)
            ones_row_f = cpool.tile([64, 128], F32, tag="ones_row_f")
            nc.vector.memset(ones_row_f[:], 1.0)
            if use_bias:
                ones_row = cpool.tile([1, 128], BF16, tag="ones_row")
                nc.vector.memset(ones_row[:], 1.0)
                bq_sb = cpool.tile([128, MQ], F32, tag="bq")
                nc.sync.dma_start(bq_sb[:], bq_t.rearrange("m p -> p m"))
                bk_sb = cpool.tile([128, MK], F32, tag="bk")
                nc.sync.dma_start(bk_sb[:], bk_t.rearrange("m p -> p m"))
                bv_sb = cpool.tile([1, 2, 512], BF16, tag="bv")
                nc.sync.dma_start(bv_sb[:], bv_t[None, :, :])
                bo_sb = cpool.tile([1, NE, 512], BF16, tag="bo")
                nc.sync.dma_start(bo_sb[:], bo_t[None, :, :])

            # persistent intermediates
            qT = keep.tile([128, MQ, TOK], BF16, tag="qT")
            kT = keep.tile([128, MK, TOK], BF16, tag="kT")
            v_sb = keep.tile([128, NBLK, HKV * D], BF16, tag="v")

            # ================= projection phase =================
            with (
                tc.tile_pool(name="xtp", bufs=1) as xt_pool,
                tc.tile_pool(name="wmov", bufs=2) as wmov_pool,
            ):
                # The n=0 V weights live in one upfront tile whose DMAs are
                # interleaved with the xt slices in consumption order: DMA
                # queues are FIFO, so anything the V loop needs early must
                # also be ISSUED early, and ring-buffered tiles cannot be
                # issued far ahead (their WAR semaphore would stall the
                # queue). Supply measures ~380 GB/s once ramped, so this
                # keeps the V projection compute-bound from ~10us.
                xt = xt_pool.tile([128, KO, TOK], BF16, tag="xt")
                with tc.tile_pool(name="wv0", bufs=1) as wv0_pool:
                    wvall = wv0_pool.tile([128, KO // 2, 2, 512], BF16,
                                          tag="wvall")
                    for kp in range(KO // 2):
                        nc.sync.dma_start(wvall[:, kp, :, :], wv_t[kp, 0])
                        nc.sync.dma_start(xt[:, 2 * kp, :], xT[2 * kp])
                        nc.sync.dma_start(xt[:, 2 * kp + 1, :], xT[2 * kp + 1])
                    nc.sync.dma_start(cos_sb[:], cos_t[:, :])
                    nc.sync.dma_start(sin_sb[:], sin_t[:, :])
                    nc.sync.dma_start(mb_sb[:], mb_t.rearrange("b p -> p b"))

                    # ---- V projection (paced by xt arrival) ----
                    with tc.tile_pool(name="psv", bufs=8, space="PSUM") as psv:
                        for n in range(2):
                            pss = [psv.tile([128, 512], F32, tag="ps",
                                            name=f"psv{n}_{mt}")
                                   for mt in range(NBLK)]
                            for kp in range(KO // 2):
                                if n == 0:
                                    wv_sb = wvall[:, kp, :, :]
                                else:
                                    wv_t2 = wmov_pool.tile(
                                        [128, 2, 512], BF16, tag="wv")
                                    nc.sync.dma_start(wv_t2[:], wv_t[kp, n])
                                    wv_sb = wv_t2[:]
                                for k2 in range(2):
                                    ko = 2 * kp + k2
                                    for mt in range(NBLK):
                                        nc.tensor.matmul(
                                            pss[mt][:],
                                            xt[:, ko, mt * 128:(mt + 1) * 128],
                                            wv_sb[:, k2, :], start=(ko == 0),
                                            stop=(ko == KO - 1 and not use_bias))
                            if use_bias:
                                for mt in range(NBLK):
                                    nc.tensor.matmul(pss[mt][:], ones_row[:],
                                                     bv_sb[:, n, :],
                                                     start=False, stop=True)
                            for mt in range(NBLK):
                                dst = v_sb[:, mt, n * 512:(n + 1) * 512]
                                nc.scalar.copy(dst[:, 0:256], pss[mt][:, 0:256])
                                nc.vector.tensor_copy(dst[:, 256:512],
                                                      pss[mt][:, 256:512])

                # ---- K/Q projections with fused RoPE eviction ----
                def rope_evict(ps, dst_ap, h, m, bias_sb):
                    ts = slice(h * 512, (h + 1) * 512)
                    qa = rope_pool.tile([128, 512], BF16, tag="qa")
                    if bias_sb is not None:
                        nc.scalar.add(qa[:], ps[:], bias_sb[:, m:m + 1])
                    else:
                        nc.scalar.copy(qa[:], ps[:])
                    qsh = rope_pool.tile([128, 512], BF16, tag="qsh")
                    nc.sync.dma_start(qsh[0:64, :], qa[64:128, :])
                    nc.sync.dma_start(qsh[64:128, :], qa[0:64, :])
                    t1 = rope_pool.tile([128, 512], BF16, tag="t1")
                    nc.vector.tensor_mul(t1[:], qa[:], cos_sb[:, ts])
                    t2 = rope_pool.tile([128, 512], BF16, tag="t2")
                    nc.vector.tensor_mul(t2[:], qsh[:], sin_sb[:, ts])
                    nc.vector.tensor_add(dst_ap, t1[:], t2[:])

                # ---- K proj, then Q proj with woven attention ----
                # The attention pipeline rides inside the Q-projection
                # stream: one attention step is emitted after each Q psum
                # chain (64 chains <-> 64 steps, offset 9 so group g's
                # attention runs while group g+1 projects). Every attention
                # operand is then ~7us old when the PE reaches it, and the
                # ACT Exp<->Reciprocal table reloads (1.3us each) hide
                # entirely behind projection matmuls. Attention output is
                # normalized in-place into qT (each slice is dead as a query
                # once its s-matmul has run), which keeps SBUF under budget.
                with (
                    tc.tile_pool(name="wqk", bufs=2) as wqk_pool,
                    tc.tile_pool(name="rope", bufs=2) as rope_pool,
                    tc.tile_pool(name="psqk", bufs=2, space="PSUM") as psqk,
                    tc.tile_pool(name="ps_s", bufs=2, space="PSUM") as ps_s,
                    tc.tile_pool(name="ps_l", bufs=2, space="PSUM") as ps_l,
                    tc.tile_pool(name="ps_pv", bufs=1, space="PSUM") as ps_pv,
                    tc.tile_pool(name="ps_bc", bufs=1, space="PSUM") as ps_bc,
                    tc.tile_pool(name="attn", bufs=6) as attn_pool,
                    tc.tile_pool(name="bcbp", bufs=2) as bcb_pool,
                    tc.tile_pool(name="rcp", bufs=3) as rc_pool,
                ):
                    for m in range(MK):
                        wsb = wqk_pool.tile([128, KO, 128], BF16, tag="w")
                        nc.sync.dma_start(wsb[:], wk_t[m])
                        for h in range(2):
                            ps = psqk.tile([128, 512], F32, tag="ps")
                            for ko in range(KO):
                                nc.tensor.matmul(
                                    ps[:], wsb[:, ko, :],
                                    xt[:, ko, h * 512:(h + 1) * 512],
                                    start=(ko == 0), stop=(ko == KO - 1))
                            rope_evict(ps, kT[:, m, h * 512:(h + 1) * 512],
                                       h, m, bk_sb if use_bias else None)

                    iters = [(g, blk) for g in range(HKV)
                             for blk in range(NBLK)]
                    NIT = len(iters)
                    LAG = 5
                    st_wT = {}
                    st_rc = {}
                    l_cur = {}

                    def op_s(i):
                        # stage A: scores + exp for iteration i
                        if i < NIT:
                            g, blk = iters[i]
                            tq = slice(blk * 128, (blk + 1) * 128)
                            s_ps = ps_s.tile([128, 512], F32, tag="s")
                            nc.tensor.matmul(
                                s_ps[:], kT[:, g, tq],
                                qT[:, g * G:(g + 1) * G, tq],
                                start=True, stop=True)
                            wT = attn_pool.tile([128, 512], BF16, tag="wT")
                            nc.scalar.activation(
                                out=wT[:], in_=s_ps[:],
                                func=mybir.ActivationFunctionType.Exp,
                                scale=SCALE, bias=mb_sb[:, blk:blk + 1])
                            st_wT[i] = wT

                    def op_l(i):
                        # stage B: row-sum; adjacent reciprocal pair per 4
                        j = i - 1
                        if 0 <= j < NIT:
                            b, r = divmod(j, 2)
                            if r == 0:
                                l_cur[b] = ps_l.tile([128, 512], F32,
                                                     tag="l", name=f"l{b}")
                            nc.tensor.matmul(
                                l_cur[b][32 * r:32 * r + 1, :], ones_col[:],
                                st_wT[j][:], start=True, stop=True)
                            if r == 1 and (b % 2 == 1 or j == NIT - 1):
                                for bb in ([b - 1, b] if b % 2 == 1 else [b]):
                                    rc = rc_pool.tile([64, 512], F32,
                                                      tag="rc", name=f"rc{bb}")
                                    _act_recip(nc, rc[:], l_cur[bb][0:64, :])
                                    st_rc[bb] = rc
                                    del l_cur[bb]

                    def op_pv(i):
                        j = i - LAG
                        if 0 <= j < NIT:
                            g, blk = iters[j]
                            pv_ps = ps_pv.tile([128, 512], F32, tag="pv")
                            nc.tensor.matmul(
                                pv_ps[:],
                                v_sb[:, blk, g * 128:(g + 1) * 128],
                                st_wT.pop(j)[:], start=True, stop=True)
                            st_wT[-j - 1] = pv_ps

                    def op_bc(i):
                        # stage C: normalize broadcast + in-place store
                        j = i - LAG
                        if 0 <= j < NIT:
                            g, blk = iters[j]
                            b, r = divmod(j, 2)
                            pv_ps = st_wT.pop(-j - 1)
                            bc_ps = ps_bc.tile([128, 512], F32, tag="bc")
                            nc.tensor.matmul(
                                bc_ps[:], ones_row_f[32 * r:32 * r + 1, :],
                                st_rc[b][32 * r:32 * r + 1, :],
                                start=True, stop=True)
                            if r == 1:
                                del st_rc[b]
                            bcb = bcb_pool.tile([128, 512], BF16, tag="bcb")
                            nc.vector.tensor_copy(bcb[:], bc_ps[:])
                            nc.vector.tensor_mul(
                                qT[:, g * G:(g + 1) * G,
                                   blk * 128:(blk + 1) * 128],
                                pv_ps[:], bcb[:])

                    def attn_step(i):
                        op_s(i)
                        op_l(i)
                        op_pv(i)
                        op_bc(i)

                    chain = 0
                    for m in range(MQ):
                        wsb = wqk_pool.tile([128, KO, 128], BF16, tag="w")
                        nc.sync.dma_start(wsb[:], wq_t[m])
                        for h in range(2):
                            # the four attention PE ops of step (chain-10)
                            # are spread inside this chain's ko loop so
                            # their LDWEIGHTS hide under the projection
                            # stream instead of clumping between chains.
                            i_w = chain - 9 if chain >= 9 else None
                            ps = psqk.tile([128, 512], F32, tag="ps")
                            for ko in range(KO):
                                nc.tensor.matmul(
                                    ps[:], wsb[:, ko, :],
                                    xt[:, ko, h * 512:(h + 1) * 512],
                                    start=(ko == 0), stop=(ko == KO - 1))
                                if i_w is not None and 0 <= i_w - 1:
                                    if ko == 7:
                                        op_s(i_w - 1)
                                    elif ko == 15:
                                        op_l(i_w - 1)
                                    elif ko == 23:
                                        op_pv(i_w - 1)
                                    elif ko == 30:
                                        op_bc(i_w - 1)
                            rope_evict(ps, qT[:, m, h * 512:(h + 1) * 512],
                                       h, m, bq_sb if use_bias else None)
                            chain += 1
                    # tail: group 7's attention alone is ACT-bound (~50%
                    # PE duty), which lets HAM re-throttle and the out
                    # projection then starts at half clock. Weave the first
                    # out-projection chains (n=0, token blocks 0-1, using
                    # the now-idle psqk banks) into the tail steps as real
                    # PE work: the tail stays dense and out-proj effectively
                    # starts during it. Blocks 0-1 of n=0 are then skipped
                    # in the main out-proj loop.
                    pssA = [psqk.tile([128, 512], F32, tag="ps",
                                      name=f"psA{t}") for t in range(2)]
                    hk_done = 0
                    tail = list(range(chain - 10, NIT + LAG))
                    for idx, i in enumerate(tail):
                        attn_step(i)
                        target = min(MQ, idx * 3)
                        if idx == len(tail) - 1:
                            target = MQ
                        while hk_done < target:
                            hp = hk_done // 2
                            wo_sb = wmov_pool.tile([128, 2, 512], BF16,
                                                   tag="wv")
                            nc.sync.dma_start(wo_sb[:], wo_t[hp, 0])
                            for k2 in range(2):
                                hk = 2 * hp + k2
                                for t in range(2):
                                    nc.tensor.matmul(
                                        pssA[t][:],
                                        qT[:, hk, t * 128:(t + 1) * 128],
                                        wo_sb[:, k2, :], start=(hk == 0),
                                        stop=(hk == MQ - 1 and not use_bias))
                            hk_done += 2
                    if use_bias:
                        for t in range(2):
                            nc.tensor.matmul(pssA[t][:], ones_row[:],
                                             bo_sb[:, 0, :],
                                             start=False, stop=True)
                    for t in range(2):
                        oeA = bcb_pool.tile([128, 512], BF16, tag="bcb",
                                            name=f"oeA{t}")
                        nc.scalar.copy(oeA[:, 0:256], pssA[t][:, 0:256])
                        nc.vector.tensor_copy(oeA[:, 256:512],
                                              pssA[t][:, 256:512])
                        nc.sync.dma_start(
                            out[t * 128:(t + 1) * 128, 0:512], oeA[:])

            # ================= out projection =================
            with (
                tc.tile_pool(name="wop", bufs=8) as wo_pool,
                tc.tile_pool(name="oe", bufs=6) as oe_pool,
                tc.tile_pool(name="psO", bufs=8, space="PSUM") as psO,
            ):
                for n in range(NE):
                    mts = list(range(2, NBLK)) if n == 0 else list(range(NBLK))
                    pss = {mt: psO.tile([128, 512], F32, tag="ps",
                                        name=f"pso{n}_{mt}")
                           for mt in mts}
                    for hp in range(MQ // 2):
                        wo_sb = wo_pool.tile([128, 2, 512], BF16, tag="wo")
                        nc.sync.dma_start(wo_sb[:], wo_t[hp, n])
                        for k2 in range(2):
                            hk = 2 * hp + k2
                            for mt in mts:
                                nc.tensor.matmul(
                                    pss[mt][:],
                                    qT[:, hk, mt * 128:(mt + 1) * 128],
                                    wo_sb[:, k2, :], start=(hk == 0),
                                    stop=(hk == MQ - 1 and not use_bias))
                    if use_bias:
                        for mt in mts:
                            nc.tensor.matmul(pss[mt][:], ones_row[:],
                                             bo_sb[:, n, :],
                                             start=False, stop=True)
                    for mt in mts:
                        oe = oe_pool.tile([128, 512], BF16, tag="oe")
                        nc.scalar.copy(oe[:, 0:256], pss[mt][:, 0:256])
                        nc.vector.tensor_copy(oe[:, 256:512],
                                              pss[mt][:, 256:512])
                        nc.sync.dma_start(
                            out[mt * 128:(mt + 1) * 128,
                                n * 512:(n + 1) * 512], oe[:])

    return _split_excess_waits(nc)


_NC_CACHE = {}


def _get_nc(use_bias: bool):
    if use_bias not in _NC_CACHE:
        _NC_CACHE[use_bias] = _build(use_bias)
    return _NC_CACHE[use_bias]


def _prepare(x, wq, bq, wk, bk, wv, bv, wo, bo, mask):
    x = np.asarray(x, np.float32)
    wq = np.asarray(wq, np.float32)
    wk = np.asarray(wk, np.float32)
    wv = np.asarray(wv, np.float32)
    wo = np.asarray(wo, np.float32)
    bq = np.asarray(bq, np.float32)
    bk = np.asarray(bk, np.float32)
    bv = np.asarray(bv, np.float32)
    bo = np.asarray(bo, np.float32)
    mask = np.asarray(mask)

    use_bias = bool(bq.any() or bk.any() or bv.any() or bo.any())

    # weight layouts (shared across cores)
    wq_t = np.ascontiguousarray(
        wq.reshape(KO, 128, MQ, 128).transpose(2, 1, 0, 3)).astype(NPBF16)
    wk_t = np.ascontiguousarray(
        wk.reshape(KO, 128, MK, 128).transpose(2, 1, 0, 3)).astype(NPBF16)
    wv_t = np.ascontiguousarray(
        wv.reshape(KO // 2, 2, 128, 2, 512).transpose(0, 3, 2, 1, 4)).astype(NPBF16)
    wo_t = np.ascontiguousarray(
        wo.reshape(MQ // 2, 2, 128, NE, 512).transpose(0, 3, 2, 1, 4)).astype(NPBF16)

    # RoPE tables (positions are global sequence positions)
    inv = 1.0 / (ROPE_BASE ** (np.arange(0, D, 2, dtype=np.float32) / D))
    pos = np.arange(S, dtype=np.float32)
    ang = pos[:, None] * inv[None, :]                      # [S, 64]
    cos_full = np.concatenate([np.cos(ang), np.cos(ang)], -1).T  # [128, S]
    sin_half = np.sin(ang).T                               # [64, S]
    sin_eff = np.concatenate([-sin_half, sin_half], 0)     # [128, S]

    shards_per_b = NCORES // B                             # 4
    in_maps = []
    for c in range(NCORES):
        b = c // shards_per_b
        s0 = (c % shards_per_b) * TOK
        xs = x[b, s0:s0 + TOK]                             # [TOK, E]
        xT_t = np.ascontiguousarray(xs.T).astype(NPBF16).reshape(KO, 128, TOK)
        mshard = mask[b, s0:s0 + TOK].reshape(NBLK, BS)
        mb = np.where(mshard, np.float32(0.0), np.float32(-80.0)).astype(np.float32)
        im = {
            "xT": xT_t,
            "wq_t": wq_t, "wk_t": wk_t, "wv_t": wv_t, "wo_t": wo_t,
            "cos_t": np.ascontiguousarray(cos_full[:, s0:s0 + TOK]).astype(NPBF16),
            "sin_t": np.ascontiguousarray(sin_eff[:, s0:s0 + TOK]).astype(NPBF16),
            "mb_t": mb,
        }
        if use_bias:
            im["bq_t"] = bq.reshape(MQ, 128).copy()
            im["bk_t"] = bk.reshape(MK, 128).copy()
            im["bv_t"] = bv.reshape(2, 512).astype(NPBF16)
            im["bo_t"] = bo.reshape(NE, 512).astype(NPBF16)
        in_maps.append(im)

    return in_maps, use_bias


def _assemble(results):
    shards_per_b = NCORES // B
    out = np.empty((B, S, E), np.float32)
    for c in range(NCORES):
        b = c // shards_per_b
        s0 = (c % shards_per_b) * TOK
        out[b, s0:s0 + TOK] = np.asarray(results[c]["out"], np.float32)
    return out


def kernel(**inputs):
    in_maps, use_bias = _prepare(**inputs)
    nc = _get_nc(use_bias)
    res = run_bass_kernel_spmd(nc, in_maps, core_ids=list(range(NCORES)))
    return _assemble(res.results)
